# revision 1
# baseline (speedup 1.0000x reference)
"""CompoundHeadAttention TRN2 kernel.

Full-input contract: kernel(**inputs) takes the unsharded tensors from
setup_inputs() and returns the full [1, 2048, 2048] float32 output.

Sharding (8 cores, tensor-parallel over the HK=8 kv heads):
  core h owns kv head h: its Wq/Wk/Wv column slice, its WG[h]/bG[h], and
  Wfc row-slice [h*256:(h+1)*256, :].  Each core computes its head's
  attention + its partial FC output [2048, 2048]; the host sums the 8
  partials and adds bfc (the "all-reduce" of the row-sharded FC).

Device-side math per core (N=2048, E=2048, D=64, G=4):
  QT  [128, n] = dup([Wq_h|Wq_h]^T q^T) + bq        (fp16 matmul, fp32 psum)
  KT  [128, s] = dup                                 (dup rows = row-tiling feed)
  V   [s, 64]  via VT matmul + PE transpose, ones column appended (M=65)
  QgT [128, n] per g-pair via WG row-tiled matmuls
  ST  [s=128, n=512] = KT_chunk^T QgT  (two row-tiled K=64 matmuls)
  PT  = exp(8*ST)  (ACT, scale folds the D**-0.5 softmax scale)
  causal mask: gpsimd affine_select zeroes PT where n < s (diagonal chunks)
  PV  [65, n] += Vones_chunk^T PT  (row 64 = softmax denominators)
  hidden = PV[0:64] * recip(PV[64])  (DVE + gpsimd partition_broadcast)
  out_partial[n, :] = hidden01^T Wfc[0:128] + hidden23^T Wfc[128:256]

Matmul dtypes: fp16 for the projections (inputs shipped as fp16),
float32r (1 cycle/row at N=512) for everything downstream.
"""

import os
import sys

import numpy as np

if "/opt/trn_rl_repo" not in sys.path and os.path.isdir("/opt/trn_rl_repo"):
    sys.path.insert(0, "/opt/trn_rl_repo")

import concourse.bass as bass  # noqa: E402
import concourse.mybir as mybir  # noqa: E402
import concourse.tile as tile  # noqa: E402
from concourse import bacc  # noqa: E402
from concourse import bass_utils  # noqa: E402

F32 = mybir.dt.float32
F32R = mybir.dt.float32r
F16 = mybir.dt.float16
AF = mybir.ActivationFunctionType

N = 2048
E = 2048
HK = 8
D = 64
G = 4
NB = 4        # 512-wide n-windows
SC_PER_NB = 4  # 128-wide s-chunks per window
NEG = -1e30


def build_program():
    nc = bacc.Bacc("TRN2", target_bir_lowering=False, debug=False,
                   enable_asserts=False)

    # ---- DRAM I/O ----
    qT = nc.dram_tensor("qT", [E, N], F16, kind="ExternalInput").ap()
    kT = nc.dram_tensor("kT", [E, N], F16, kind="ExternalInput").ap()
    vT = nc.dram_tensor("vT", [E, N], F16, kind="ExternalInput").ap()
    # weight chunk layout: [128, 16*M] — e-chunk ec occupies cols [M*ec, M*ec+M)
    wq = nc.dram_tensor("wq", [128, 16 * 128], F16, kind="ExternalInput").ap()
    wk = nc.dram_tensor("wk", [128, 16 * 128], F16, kind="ExternalInput").ap()
    wv = nc.dram_tensor("wv", [128, 16 * 64], F16, kind="ExternalInput").ap()
    bq2 = nc.dram_tensor("bq2", [128, 1], F32, kind="ExternalInput").ap()
    bk2 = nc.dram_tensor("bk2", [128, 1], F32, kind="ExternalInput").ap()
    bvv = nc.dram_tensor("bvv", [64, 1], F32, kind="ExternalInput").ap()
    wg = nc.dram_tensor("wg", [128, 256], F32R, kind="ExternalInput").ap()
    bg01 = nc.dram_tensor("bg01", [128, 1], F32, kind="ExternalInput").ap()
    bg23 = nc.dram_tensor("bg23", [128, 1], F32, kind="ExternalInput").ap()
    wfc = nc.dram_tensor("wfc", [256, E], F32R, kind="ExternalInput").ap()
    ident = nc.dram_tensor("ident", [128, 128], F32, kind="ExternalInput").ap()
    out = nc.dram_tensor("out", [N, E], F32, kind="ExternalOutput").ap()

    with tile.TileContext(nc) as tc:
        build_tile_kernel(tc, qT=qT, kT=kT, vT=vT, wq=wq, wk=wk, wv=wv,
                          bq2=bq2, bk2=bk2, bvv=bvv, wg=wg, bg01=bg01,
                          bg23=bg23, wfc=wfc, ident=ident, out=out)
    nc.compile()
    return nc


def build_tile_kernel(tc, *, qT, kT, vT, wq, wk, wv, bq2, bk2, bvv, wg,
                      bg01, bg23, wfc, ident, out):
    nc = tc.nc

    import contextlib
    ctx = contextlib.ExitStack()
    ctx.__enter__()
    cp = ctx.enter_context(tc.tile_pool(name="persist", bufs=1))

    def ptile(shape, dtype, name):
        return cp.tile(shape, dtype, tag=name, name=name)

    # ---- persistent constants in SBUF ----
    wq_sb = ptile([128, 16 * 128], F16, "wq_sb")
    wk_sb = ptile([128, 16 * 128], F16, "wk_sb")
    wv_sb = ptile([128, 16 * 64], F16, "wv_sb")
    wg_sb = ptile([128, 256], F32R, "wg_sb")
    wfc0_sb = ptile([128, E], F32R, "wfc0_sb")
    wfc1_sb = ptile([128, E], F32R, "wfc1_sb")
    id_sb = ptile([128, 128], F32, "id_sb")
    bq_sb = ptile([128, 1], F32, "bq_sb")
    bk_sb = ptile([128, 1], F32, "bk_sb")
    bv_sb = ptile([64, 1], F32, "bv_sb")
    bg01_sb = ptile([128, 1], F32, "bg01_sb")
    bg23_sb = ptile([128, 1], F32, "bg23_sb")
    ones_sb = ptile([128, 1], F32, "ones_sb")
    nc.vector.memset(ones_sb[:], 1.0)

    # only wq is needed before the first q rows can be consumed; the
    # remaining consts are interleaved into the Q loop (emit_consts).
    nc.sync.dma_start(wq_sb[:], wq[:])

    # per-window persistent activations
    qt_w = [ptile([128, 512], F32R, f"qt{j}") for j in range(NB)]
    kt_w = [ptile([128, 512], F32R, f"kt{j}") for j in range(NB)]
    vo_w = [ptile([128, 4 * 65], F32R, f"vo{j}") for j in range(NB)]
    qg01_w = [ptile([128, 512], F32R, f"qg01_{j}") for j in range(NB)]
    qg23_w = [ptile([128, 512], F32R, f"qg23_{j}") for j in range(NB)]
    hid01_w = [ptile([128, 512], F32R, f"hid01_{j}") for j in range(NB)]
    hid23_w = [ptile([128, 512], F32R, f"hid23_{j}") for j in range(NB)]

    with ctx:
        in_pool = ctx.enter_context(tc.tile_pool(name="in_pool", bufs=9))
        vt_pool = ctx.enter_context(tc.tile_pool(name="vt_pool", bufs=2))
        pt_pool = ctx.enter_context(tc.tile_pool(name="pt_pool", bufs=4))
        rec_pool = ctx.enter_context(tc.tile_pool(name="rec_pool", bufs=2))
        fco_pool = ctx.enter_context(tc.tile_pool(name="fco_pool", bufs=2))
        misc_ps = ctx.enter_context(
            tc.tile_pool(name="misc_ps", bufs=2, space="PSUM"))
        st_ps = ctx.enter_context(
            tc.tile_pool(name="st_ps", bufs=2, space="PSUM"))
        pv_ps = ctx.enter_context(
            tc.tile_pool(name="pv_ps", bufs=2, space="PSUM"))

        def emit_proj(P):
            """projections + G for window pair P ({0,1} or {2,3})"""
            pcol = bass.ds(P * 1024, 1024)
            wins = (2 * P, 2 * P + 1)

            # Q projection (both windows), dup'd output partitions
            q0_ps = misc_ps.tile([128, 512], F32, tag="mm", name="q0_ps")
            q1_ps = misc_ps.tile([128, 512], F32, tag="mm", name="q1_ps")
            for ec in range(16):
                q_in = in_pool.tile([128, 1024], F16, tag="qin", name="q_in")
                nc.sync.dma_start(q_in[:], qT[bass.ts(ec, 128), pcol])
                w = wq_sb[:, bass.ts(ec, 128)]
                nc.tensor.matmul(q0_ps[:], w, q_in[:, 0:512],
                                 start=(ec == 0), stop=(ec == 15))
                nc.tensor.matmul(q1_ps[:], w, q_in[:, 512:1024],
                                 start=(ec == 0), stop=(ec == 15))
                yield
            nc.scalar.activation(qt_w[wins[0]][:], q0_ps[:], AF.Identity, bias=bq_sb[:])
            nc.scalar.activation(qt_w[wins[1]][:], q1_ps[:], AF.Identity, bias=bq_sb[:])

            # K projection (both windows)
            k0_ps = misc_ps.tile([128, 512], F32, tag="mm", name="k0_ps")
            k1_ps = misc_ps.tile([128, 512], F32, tag="mm", name="k1_ps")
            for ec in range(16):
                k_in = in_pool.tile([128, 1024], F16, tag="kin", name="k_in")
                nc.sync.dma_start(k_in[:], kT[bass.ts(ec, 128), pcol])
                w = wk_sb[:, bass.ts(ec, 128)]
                nc.tensor.matmul(k0_ps[:], w, k_in[:, 0:512],
                                 start=(ec == 0), stop=(ec == 15))
                nc.tensor.matmul(k1_ps[:], w, k_in[:, 512:1024],
                                 start=(ec == 0), stop=(ec == 15))
                yield
            nc.scalar.activation(kt_w[wins[0]][:], k0_ps[:], AF.Identity, bias=bk_sb[:])
            nc.scalar.activation(kt_w[wins[1]][:], k1_ps[:], AF.Identity, bias=bk_sb[:])

            # V projection: VT then PE-transpose to V (+ ones column)
            v0_ps = misc_ps.tile([64, 512], F32, tag="mm", name="v0_ps")
            v1_ps = misc_ps.tile([64, 512], F32, tag="mm", name="v1_ps")
            for ec in range(16):
                v_in = in_pool.tile([128, 1024], F16, tag="vin", name="v_in")
                nc.sync.dma_start(v_in[:], vT[bass.ts(ec, 128), pcol])
                w = wv_sb[:, bass.ts(ec, 64)]
                nc.tensor.matmul(v0_ps[:], w, v_in[:, 0:512],
                                 start=(ec == 0), stop=(ec == 15))
                nc.tensor.matmul(v1_ps[:], w, v_in[:, 512:1024],
                                 start=(ec == 0), stop=(ec == 15))
                yield
            for wi, v_ps in ((wins[0], v0_ps), (wins[1], v1_ps)):
                vt_sb = vt_pool.tile([64, 512], F32, tag="vt", name="vt_sb")
                nc.scalar.activation(vt_sb[:], v_ps[:], AF.Identity, bias=bv_sb[:])
                tr_ps = misc_ps.tile([128, 256], F32, tag="mm", name="tr_ps")
                for t in range(4):
                    nc.tensor.transpose(tr_ps[:, bass.ts(t, 64)],
                                        vt_sb[:, bass.ts(t, 128)],
                                        id_sb[0:64, 0:64])
                for t in range(4):
                    nc.vector.tensor_copy(vo_w[wi][:, t * 65:t * 65 + 64],
                                          tr_ps[:, bass.ts(t, 64)])
                    nc.vector.tensor_copy(
                        vo_w[wi][:, t * 65 + 64:t * 65 + 65], ones_sb[:])

            # G transform per window, row-tiled pair01 / pair23
            for wi in wins:
                g01_ps = misc_ps.tile([128, 512], F32, tag="mm", name="g01_ps")
                g23_ps = misc_ps.tile([128, 512], F32, tag="mm", name="g23_ps")
                nc.tensor.matmul(g01_ps[:], wg_sb[0:64, 0:128],
                                 qt_w[wi][0:64, :], start=True, stop=True)
                nc.tensor.matmul(g23_ps[:], wg_sb[64:128, 128:256],
                                 qt_w[wi][64:128, :], start=True, stop=True)
                nc.scalar.activation(qg01_w[wi][:], g01_ps[:], AF.Identity,
                                     bias=bg01_sb[:])
                nc.scalar.activation(qg23_w[wi][:], g23_ps[:], AF.Identity,
                                     bias=bg23_sb[:])
                yield

        def emit_attn(j):
            klast = 4 * j + 3
            for (qg, hid) in ((qg01_w[j], hid01_w[j]),
                              (qg23_w[j], hid23_w[j])):
                pv_a = pv_ps.tile([65, 512], F32, tag="pv", name="pv_a")
                pv_b = pv_ps.tile([65, 512], F32, tag="pv", name="pv_b")
                for k in range(klast + 1):
                    kt_c = kt_w[k // 4]
                    ks = bass.ts(k % 4, 128)
                    # causal trim: diagonal chunk k covers n-cols [off, 512).
                    # i=3 is padded to 256 wide: float32r matmuls below 256
                    # moving cols run at 1/4 rate, so N=128 costs as much as
                    # N=512 while N=256 costs half.
                    i = k - 4 * j
                    off = max(0, 128 * i)
                    if off == 384:
                        off = 256
                    st = st_ps.tile([128, 1024], F32, tag="st", name="st")
                    nc.tensor.matmul(st[:, off:512], kt_c[0:64, ks],
                                     qg[0:64, off:512], start=True, stop=True)
                    nc.tensor.matmul(st[:, 512 + off:1024], kt_c[64:128, ks],
                                     qg[64:128, off:512],
                                     start=True, stop=True)
                    pt = pt_pool.tile([128, 1024], F32R, tag="pt", name="pt")
                    st3 = st[:].rearrange("p (h c) -> p h c", c=512)
                    pt3 = pt[:].rearrange("p (h c) -> p h c", c=512)
                    nc.scalar.activation(pt3[:, :, off:512],
                                         st3[:, :, off:512],
                                         AF.Exp, scale=8.0)
                    if i >= 0:
                        # mask region [off, 128*i+128): keep where global
                        # col >= s + 128*i, i.e. local c' >= s + (128*i-off)
                        mw = 128 * i + 128 - off
                        nc.gpsimd.affine_select(
                            out=pt3[:, :, off:off + mw],
                            in_=pt3[:, :, off:off + mw],
                            compare_op=mybir.AluOpType.is_ge,
                            fill=0.0, base=-(128 * i - off),
                            pattern=[[0, 2], [1, mw]],
                            channel_multiplier=-1)
                    vo_c = vo_w[k // 4]
                    vsl = vo_c[:, (k % 4) * 65:(k % 4) * 65 + 65]
                    nc.tensor.matmul(pv_a[:, off:512], vsl, pt[:, off:512],
                                     start=(k == 0), stop=(k == klast))
                    nc.tensor.matmul(pv_b[:, off:512], vsl,
                                     pt[:, 512 + off:1024],
                                     start=(k == 0), stop=(k == klast))
                    yield
                # normalize: hidden[g-half] = pv[0:64] * 1/pv[64]
                for half, pv in ((0, pv_a), (1, pv_b)):
                    rec = rec_pool.tile([1, 512], F32, tag="rec", name="rec")
                    nc.vector.reciprocal(rec[:], pv[64:65, :])
                    recr = rec_pool.tile([64, 512], F32, tag="recr",
                                         name="recr")
                    nc.gpsimd.partition_broadcast(recr[:], rec[:])
                    nc.vector.tensor_mul(hid[half * 64:half * 64 + 64, :],
                                         pv[0:64, :], recr[:])

        def emit_fc(j):
            for m in range(4):
                msl = bass.ts(m, 128)
                stage = fco_pool.tile([128, 2048], F32, tag="fco",
                                      name="stage")
                for eo in range(4):
                    fc_ps = misc_ps.tile([128, 512], F32, tag="mm",
                                         name="fc_ps")
                    nc.tensor.matmul(fc_ps[:], hid01_w[j][:, msl],
                                     wfc0_sb[:, bass.ts(eo, 512)],
                                     start=True, stop=False)
                    nc.tensor.matmul(fc_ps[:], hid23_w[j][:, msl],
                                     wfc1_sb[:, bass.ts(eo, 512)],
                                     start=False, stop=True)
                    nc.vector.tensor_copy(stage[:, bass.ts(eo, 512)],
                                          fc_ps[:])
                    yield
                nc.sync.dma_start(
                    out[512 * j + 128 * m: 512 * j + 128 * m + 128, :],
                    stage[:])

        def emit_consts():
            for dst, srcap in ((wk_sb, wk), (wv_sb, wv), (bq_sb, bq2),
                               (bk_sb, bk2), (bv_sb, bvv), (wg_sb, wg),
                               (id_sb, ident), (bg01_sb, bg01),
                               (bg23_sb, bg23)):
                nc.sync.dma_start(dst[:], srcap[:])
                yield

        from itertools import chain as ichain

        def drain(g):
            for _ in g:
                pass

        def rr(pairs):
            """round-robin emission: [(generator, steps_per_turn)]"""
            live = [[g, w] for g, w in pairs]
            while live:
                for gw in list(live):
                    g, w = gw
                    try:
                        for _ in range(w):
                            next(g)
                    except StopIteration:
                        live.remove(gw)

        def emit_wfc():
            nc.sync.dma_start(wfc0_sb[:], wfc[0:128, :])
            nc.sync.dma_start(wfc1_sb[:], wfc[128:256, :])
            yield

        # Phase A: pair-0 projections (DMA-bound ramp); remaining consts
        # trickle in between the first q-row loads
        rr([(emit_proj(0), 1), (emit_consts(), 1)])
        # Phase B: window-0 attention interleaved with pair-1 projections
        # (DMA hides under ACT-bound attention)
        rr([(emit_attn(0), 1), (ichain(emit_proj(1), emit_wfc()), 2)])
        # Later windows: attention with FC of completed windows as PE filler
        rr([(emit_attn(1), 1), (emit_fc(0), 1)])
        rr([(emit_attn(2), 1), (emit_fc(1), 1)])
        rr([(emit_attn(3), 1), (emit_fc(2), 1)])
        drain(emit_fc(3))


def shard_inputs(inputs):
    """full inputs -> list of 8 per-core in_maps (numpy, device layouts)"""
    f16 = np.float16
    f32 = np.float32
    q = np.asarray(inputs["q"], f32)[0]
    k = np.asarray(inputs["k"], f32)[0]
    v = np.asarray(inputs["v"], f32)[0]
    Wq = np.asarray(inputs["Wq"], f32)
    Wk = np.asarray(inputs["Wk"], f32)
    Wv = np.asarray(inputs["Wv"], f32)
    bq = np.asarray(inputs["bq"], f32)
    bk = np.asarray(inputs["bk"], f32)
    bv = np.asarray(inputs["bv"], f32)
    WG = np.asarray(inputs["WG"], f32)
    bG = np.asarray(inputs["bG"], f32)
    Wfc = np.asarray(inputs["Wfc"], f32)

    qT = np.ascontiguousarray(q.T.astype(f16))
    kT = np.ascontiguousarray(k.T.astype(f16))
    vT = np.ascontiguousarray(v.T.astype(f16))
    ident = np.eye(128, dtype=f32)

    def chunked(w):
        # [E, M] -> [128, 16*M]: e-chunk ec at cols [M*ec, M*ec+M)
        M = w.shape[1]
        return np.ascontiguousarray(
            w.reshape(16, 128, M).transpose(1, 0, 2).reshape(128, 16 * M))

    maps = []
    for h in range(HK):
        sl = slice(h * D, (h + 1) * D)
        wq_h = Wq[:, sl]
        wk_h = Wk[:, sl]
        wv_h = Wv[:, sl]
        m = {
            "qT": qT, "kT": kT, "vT": vT,
            "wq": chunked(np.concatenate([wq_h, wq_h], 1)).astype(f16),
            "wk": chunked(np.concatenate([wk_h, wk_h], 1)).astype(f16),
            "wv": chunked(wv_h).astype(f16),
            "bq2": np.concatenate([bq[sl], bq[sl]]).reshape(128, 1).copy(),
            "bk2": np.concatenate([bk[sl], bk[sl]]).reshape(128, 1).copy(),
            "bvv": bv[sl].reshape(64, 1).copy(),
            "wg": np.concatenate([WG[h], WG[h]], 0).copy(),  # [128, 256]
            "bg01": bG[h, 0:128].reshape(128, 1).copy(),
            "bg23": bG[h, 128:256].reshape(128, 1).copy(),
            "wfc": Wfc[h * 256:(h + 1) * 256, :].copy(),
            "ident": ident,
        }
        maps.append(m)
    return maps


_compiled = None
last_results = None


def get_compiled():
    global _compiled
    if _compiled is None:
        _compiled = build_program()
    return _compiled


def kernel(**inputs):
    global last_results
    nc = get_compiled()
    in_maps = shard_inputs(inputs)
    last_results = bass_utils.run_bass_kernel_spmd(
        nc, in_maps, core_ids=list(range(8)))
    bfc = np.asarray(inputs["bfc"], np.float32)
    acc = np.zeros((N, E), np.float64)
    for res in last_results.results:
        acc += res["out"].astype(np.float64)
    full = (acc + bfc[None, :].astype(np.float64)).astype(np.float32)
    return full.reshape(1, N, E)



# revision 10
# speedup vs baseline: 1.2891x; 1.2891x over previous
"""CompoundHeadAttention TRN2 kernel (v2 — software-pipelined schedule).

Full-input contract: kernel(**inputs) takes the unsharded tensors from
setup_inputs() and returns the full [1, 2048, 2048] float32 output.

Sharding (8 cores, tensor-parallel over the HK=8 kv heads):
  core h owns kv head h: its Wq/Wk/Wv column slice, its WG[h]/bG[h], and
  Wfc row-slice [h*256:(h+1)*256, :].  Each core computes its head's
  attention + its partial FC output [2048, 2048]; the host sums the 8
  partials and adds bfc (the "all-reduce" of the row-sharded FC).

v2 schedule (vs v1): the PE stream is kept dense — scores (ST) run two
chunks ahead of the PV consumer so the ACT exp latency never stalls the
in-order PE queue; projection/G/FC matmuls are interleaved into the
attention stream as filler at a per-window rate; softmax denominators
use reciprocal_approx_fast (0.7us vs 4us); input DMAs are batched
(2 e-chunks per instr) and issued from both the Sync and ACT queues.
"""

import os
import sys
from collections import deque

import numpy as np

if "/opt/trn_rl_repo" not in sys.path and os.path.isdir("/opt/trn_rl_repo"):
    sys.path.insert(0, "/opt/trn_rl_repo")

import concourse.bass as bass  # noqa: E402
import concourse.mybir as mybir  # noqa: E402
import concourse.tile as tile  # noqa: E402
from concourse import bacc  # noqa: E402
from concourse import bass_utils  # noqa: E402

F32 = mybir.dt.float32
F32R = mybir.dt.float32r
F16 = mybir.dt.float16
AF = mybir.ActivationFunctionType

N = 2048
E = 2048
HK = 8
D = 64
G = 4
NB = 4         # 512-wide n-windows
FILL_RATE = [6, 3, 2, 1]   # filler pops per attention chunk-slot, per window


def build_program():
    nc = bacc.Bacc("TRN2", target_bir_lowering=False, debug=False,
                   enable_asserts=False)

    qT = nc.dram_tensor("qT", [E, N], F16, kind="ExternalInput").ap()
    kT = nc.dram_tensor("kT", [E, N], F16, kind="ExternalInput").ap()
    vT = nc.dram_tensor("vT", [E, N], F16, kind="ExternalInput").ap()
    # weight chunk layout: [128, 16*M] — e-chunk ec occupies cols [M*ec, M*ec+M)
    wq = nc.dram_tensor("wq", [128, 16 * 128], F16, kind="ExternalInput").ap()
    wk = nc.dram_tensor("wk", [128, 16 * 128], F16, kind="ExternalInput").ap()
    wv = nc.dram_tensor("wv", [128, 16 * 64], F16, kind="ExternalInput").ap()
    bq2 = nc.dram_tensor("bq2", [128, 1], F32, kind="ExternalInput").ap()
    bk2 = nc.dram_tensor("bk2", [128, 1], F32, kind="ExternalInput").ap()
    bvv = nc.dram_tensor("bvv", [64, 1], F32, kind="ExternalInput").ap()
    wg = nc.dram_tensor("wg", [128, 256], F32R, kind="ExternalInput").ap()
    bg01 = nc.dram_tensor("bg01", [128, 1], F32, kind="ExternalInput").ap()
    bg23 = nc.dram_tensor("bg23", [128, 1], F32, kind="ExternalInput").ap()
    wfc = nc.dram_tensor("wfc", [256, E], F32R, kind="ExternalInput").ap()
    ident = nc.dram_tensor("ident", [128, 128], F32, kind="ExternalInput").ap()
    out = nc.dram_tensor("out", [N, E], F32, kind="ExternalOutput").ap()

    with tile.TileContext(nc) as tc:
        build_tile_kernel(tc, qT=qT, kT=kT, vT=vT, wq=wq, wk=wk, wv=wv,
                          bq2=bq2, bk2=bk2, bvv=bvv, wg=wg, bg01=bg01,
                          bg23=bg23, wfc=wfc, ident=ident, out=out)
    nc.compile()
    return nc


def build_tile_kernel(tc, *, qT, kT, vT, wq, wk, wv, bq2, bk2, bvv, wg,
                      bg01, bg23, wfc, ident, out):
    nc = tc.nc

    import contextlib
    ctx = contextlib.ExitStack()
    ctx.__enter__()
    cp = ctx.enter_context(tc.tile_pool(name="persist", bufs=1))

    def ptile(shape, dtype, name):
        return cp.tile(shape, dtype, tag=name, name=name)

    # ---- persistent constants / state in SBUF ----
    wq_sb = ptile([128, 16 * 128], F16, "wq_sb")
    wk_sb = ptile([128, 16 * 128], F16, "wk_sb")
    wv_sb = ptile([128, 16 * 64], F16, "wv_sb")
    wg_sb = ptile([128, 256], F32R, "wg_sb")
    wfc0_sb = ptile([128, E], F32R, "wfc0_sb")
    wfc1_sb = ptile([128, E], F32R, "wfc1_sb")
    id_sb = ptile([128, 128], F32, "id_sb")
    bq_sb = ptile([128, 1], F32, "bq_sb")
    bk_sb = ptile([128, 1], F32, "bk_sb")
    bv_sb = ptile([64, 1], F32, "bv_sb")
    bg01_sb = ptile([128, 1], F32, "bg01_sb")
    bg23_sb = ptile([128, 1], F32, "bg23_sb")
    ones_sb = ptile([128, 1], F32, "ones_sb")
    warm_sb = ptile([1, 1], F32, "warm_sb")

    kt_w = [ptile([128, 512], F32R, f"kt{j}") for j in range(NB)]
    vo_w = [ptile([128, 4 * 65], F32R, f"vo{j}") for j in range(NB)]

    with ctx:
        in_pool = ctx.enter_context(tc.tile_pool(name="in_pool", bufs=8))
        qt_pool = ctx.enter_context(tc.tile_pool(name="qt_pool", bufs=2))
        qg_pool = ctx.enter_context(tc.tile_pool(name="qg_pool", bufs=2))
        hid_pool = ctx.enter_context(tc.tile_pool(name="hid_pool", bufs=2))
        vt_pool = ctx.enter_context(tc.tile_pool(name="vt_pool", bufs=2))
        pt_pool = ctx.enter_context(tc.tile_pool(name="pt_pool", bufs=4))
        rec_pool = ctx.enter_context(tc.tile_pool(name="rec_pool", bufs=2))
        fco_pool = ctx.enter_context(tc.tile_pool(name="fco_pool", bufs=2))
        misc_ps = ctx.enter_context(
            tc.tile_pool(name="misc_ps", bufs=2, space="PSUM"))
        st_ps = ctx.enter_context(
            tc.tile_pool(name="st_ps", bufs=2, space="PSUM"))
        pv_ps = ctx.enter_context(
            tc.tile_pool(name="pv_ps", bufs=2, space="PSUM"))

        # ---------- shared state set as emission progresses ----------
        in_tiles = {}    # (tensor, pair, batch) -> sbuf tile [128, 2048]
        qg01_w = [None] * NB
        qg23_w = [None] * NB
        hid01_w = [None] * NB
        hid23_w = [None] * NB

        filler = deque()

        def fill(n):
            c = 0
            while filler and c < n:
                filler.popleft()()
                c += 1

        def drain():
            while filler:
                filler.popleft()()

        # ---------- DMA emission helpers ----------
        TSRC = {"q": (qT, "qin"), "k": (kT, "kin"), "v": (vT, "vin")}

        def emit_in_dma(t, P, b, eng):
            src_t, tag = TSRC[t]
            ti = in_pool.tile([128, 2048], F16, tag=tag, name=f"{t}in")
            src = src_t[bass.ds(256 * b, 256), bass.ds(P * 1024, 1024)]
            eng.dma_start(ti[:].rearrange("p (c n) -> p c n", c=2),
                          src.rearrange("(c p) n -> p c n", p=128))
            in_tiles[(t, P, b)] = ti

        # ---------- projection emission (per tensor, per window) ----------
        def make_proj_closures(t, j):
            """16 matmuls (8 batch-closures) + 1 bias closure for tensor t,
            window j. Sets qt/kt/vt state."""
            P, h = j // 2, j % 2
            cell = {}

            def mk_mm(b):
                def go():
                    if b == 0:
                        rows = 64 if t == "v" else 128
                        cell["ps"] = misc_ps.tile([rows, 512], F32, tag="mm",
                                                  name=f"{t}_ps")
                    w_sb = {"q": wq_sb, "k": wk_sb, "v": wv_sb}[t]
                    M = 64 if t == "v" else 128
                    ps = cell["ps"]
                    for c in range(2):
                        ec = 2 * b + c
                        mv = in_tiles[(t, P, b)][
                            :, 1024 * c + 512 * h: 1024 * c + 512 * h + 512]
                        nc.tensor.matmul(ps[:], w_sb[:, bass.ts(ec, M)], mv,
                                         start=(ec == 0), stop=(ec == 15))
                return go

            def bias():
                ps = cell["ps"]
                if t == "q":
                    qt = qt_pool.tile([128, 512], F32R, tag="qt", name="qt")
                    nc.scalar.activation(qt[:], ps[:], AF.Identity,
                                         bias=bq_sb[:])
                    cell["qt"] = qt
                elif t == "k":
                    nc.scalar.activation(kt_w[j][:], ps[:], AF.Identity,
                                         bias=bk_sb[:])
                else:
                    vt = vt_pool.tile([64, 512], F32, tag="vt", name="vt")
                    nc.scalar.activation(vt[:], ps[:], AF.Identity,
                                         bias=bv_sb[:])
                    cell["vt"] = vt

            return [mk_mm(b) for b in range(8)] + [bias], cell

        def make_vpath_closures(j, vcell):
            """PE transposes + DVE copies: VT -> vo_w[j] data columns."""
            tr_cell = {}

            def tr():
                tr_ps = misc_ps.tile([128, 256], F32, tag="mm", name="tr_ps")
                for t4 in range(4):
                    nc.tensor.transpose(tr_ps[:, bass.ts(t4, 64)],
                                        vcell["vt"][:, bass.ts(t4, 128)],
                                        id_sb[0:64, 0:64])
                tr_cell["tr"] = tr_ps

            def cp_out():
                vo3 = vo_w[j][:].rearrange("p (t c) -> p t c", c=65)
                for t4 in range(4):
                    nc.vector.tensor_copy(vo3[:, t4, 0:64],
                                          tr_cell["tr"][:, bass.ts(t4, 64)])

            return [tr, cp_out]

        def make_g_closure(j, qcell):
            def go():
                g01 = misc_ps.tile([128, 512], F32, tag="mm", name="g01_ps")
                nc.tensor.matmul(g01[:], wg_sb[0:64, 0:128],
                                 qcell["qt"][0:64, :], start=True, stop=True)
                g23 = misc_ps.tile([128, 512], F32, tag="mm", name="g23_ps")
                nc.tensor.matmul(g23[:], wg_sb[64:128, 128:256],
                                 qcell["qt"][64:128, :], start=True, stop=True)
                qg01 = qg_pool.tile([128, 512], F32R, tag="qg01", name="qg01")
                qg23 = qg_pool.tile([128, 512], F32R, tag="qg23", name="qg23")
                nc.scalar.activation(qg01[:], g01[:], AF.Identity,
                                     bias=bg01_sb[:])
                nc.scalar.activation(qg23[:], g23[:], AF.Identity,
                                     bias=bg23_sb[:])
                qg01_w[j] = qg01
                qg23_w[j] = qg23
            return go

        def push_window_feed(j):
            """Queue proj+G for window j as filler closures.  For j==1,
            interleave the pair-1 input DMAs right behind the proj closure
            that frees each input buffer."""
            def extend_interleaved(t, cls):
                mms, bias = cls[:8], cls[8]
                for b, mm in enumerate(mms):
                    filler.append(mm)
                    if j == 1:
                        filler.append(
                            lambda t=t, b=b: emit_in_dma(t, 1, b, nc.sync))
                filler.append(bias)

            qcl, qcell = make_proj_closures("q", j)
            extend_interleaved("q", qcl)
            kcl, _ = make_proj_closures("k", j)
            extend_interleaved("k", kcl)
            vcl, vcell = make_proj_closures("v", j)
            extend_interleaved("v", vcl)
            filler.extend(make_vpath_closures(j, vcell))
            filler.append(make_g_closure(j, qcell))

        # ---------- FC emission ----------
        def make_fc_closures(j):
            cls = []
            for m in range(4):
                cell = {}
                for eo in range(4):
                    def go(m=m, eo=eo, cell=cell):
                        if eo == 0:
                            cell["stage"] = fco_pool.tile(
                                [128, 2048], F32, tag="stage", name="stage")
                        fc_ps = misc_ps.tile([128, 512], F32, tag="mm",
                                             name="fc_ps")
                        nc.tensor.matmul(fc_ps[:],
                                         hid01_w[j][:, bass.ts(m, 128)],
                                         wfc0_sb[:, bass.ts(eo, 512)],
                                         start=True, stop=False)
                        nc.tensor.matmul(fc_ps[:],
                                         hid23_w[j][:, bass.ts(m, 128)],
                                         wfc1_sb[:, bass.ts(eo, 512)],
                                         start=False, stop=True)
                        nc.vector.tensor_copy(
                            cell["stage"][:, bass.ts(eo, 512)], fc_ps[:])
                        if eo == 3:
                            nc.sync.dma_start(
                                out[512 * j + 128 * m: 512 * j + 128 * m + 128,
                                    :],
                                cell["stage"][:])
                    cls.append(go)
            return cls

        # ---------- attention emission ----------
        def emit_window_attn(j):
            K = 4 * j + 4
            for pair, qg_of in ((0, qg01_w), (1, qg23_w)):
                qg = qg_of[j]
                pv_a = pv_ps.tile([65, 512], F32, tag="pv", name="pv_a")
                pv_b = pv_ps.tile([65, 512], F32, tag="pv", name="pv_b")
                pts = {}

                def st_step(k):
                    kt_c = kt_w[k // 4]
                    ks = bass.ts(k % 4, 128)
                    i = k - 4 * j
                    off = max(0, 128 * i)
                    if off == 384:
                        off = 256
                    st = st_ps.tile([128, 1024], F32, tag="st", name="st")
                    nc.tensor.matmul(st[:, off:512], kt_c[0:64, ks],
                                     qg[0:64, off:512], start=True, stop=True)
                    nc.tensor.matmul(st[:, 512 + off:1024], kt_c[64:128, ks],
                                     qg[64:128, off:512],
                                     start=True, stop=True)
                    pt = pt_pool.tile([128, 1024], F32R, tag="pt", name="pt")
                    st3 = st[:].rearrange("p (g c) -> p g c", c=512)
                    pt3 = pt[:].rearrange("p (g c) -> p g c", c=512)
                    nc.scalar.activation(pt3[:, :, off:512],
                                         st3[:, :, off:512],
                                         AF.Exp, scale=8.0)
                    if i >= 0:
                        mw = 128 * i + 128 - off
                        nc.gpsimd.affine_select(
                            out=pt3[:, :, off:off + mw],
                            in_=pt3[:, :, off:off + mw],
                            compare_op=mybir.AluOpType.is_ge,
                            fill=0.0, base=-(128 * i - off),
                            pattern=[[0, 2], [1, mw]],
                            channel_multiplier=-1)
                    pts[k] = (pt, off)

                def pv_step(k):
                    pt, off = pts.pop(k)
                    vo_c = vo_w[k // 4]
                    vsl = vo_c[:, (k % 4) * 65:(k % 4) * 65 + 65]
                    nc.tensor.matmul(pv_a[:, off:512], vsl, pt[:, off:512],
                                     start=(k == 0), stop=(k == K - 1))
                    nc.tensor.matmul(pv_b[:, off:512], vsl,
                                     pt[:, 512 + off:1024],
                                     start=(k == 0), stop=(k == K - 1))

                st_step(0)
                if K > 1:
                    st_step(1)
                for k in range(K):
                    if k + 2 < K:
                        st_step(k + 2)
                    # extra filler at the pair start covers the previous
                    # pair's normalize chain before pv psum reuse
                    fill(FILL_RATE[j] + (2 if k < 2 else 0))
                    pv_step(k)

                # normalize: hid[g-half] = pv[0:64] * (1/pv[64])
                if pair == 0:
                    hid = hid_pool.tile([128, 512], F32R, tag="hid01",
                                        name="hid01")
                    hid01_w[j] = hid
                else:
                    hid = hid_pool.tile([128, 512], F32R, tag="hid23",
                                        name="hid23")
                    hid23_w[j] = hid
                den_a = rec_pool.tile([1, 512], F32, tag="den", name="den_a")
                nc.vector.tensor_copy(den_a[:], pv_a[64:65, :])
                rec_a = rec_pool.tile([1, 512], F32, tag="rec", name="rec_a")
                nc.vector.reciprocal_approx_fast(rec_a[:], den_a[:])
                recr_a = rec_pool.tile([64, 512], F32, tag="recr",
                                       name="recr_a")
                nc.gpsimd.partition_broadcast(recr_a[:], rec_a[:])
                den_b = rec_pool.tile([1, 512], F32, tag="den", name="den_b")
                nc.vector.tensor_copy(den_b[:], pv_b[64:65, :])
                rec_b = rec_pool.tile([1, 512], F32, tag="rec", name="rec_b")
                nc.vector.reciprocal_approx_fast(rec_b[:], den_b[:])
                recr_b = rec_pool.tile([64, 512], F32, tag="recr",
                                       name="recr_b")
                nc.gpsimd.partition_broadcast(recr_b[:], rec_b[:])
                nc.vector.tensor_mul(hid[0:64, :], pv_a[0:64, :], recr_a[:])
                nc.vector.tensor_mul(hid[64:128, :], pv_b[0:64, :],
                                     recr_b[:])

        # ================= prologue =================
        nc.vector.memset(ones_sb[:], 1.0)
        nc.scalar.activation(warm_sb[:], ones_sb[0:1, :], AF.Exp, scale=1.0)
        for j in range(NB):
            for t4 in range(4):
                nc.vector.tensor_copy(
                    vo_w[j][:, t4 * 65 + 64: t4 * 65 + 65], ones_sb[:])

        nc.sync.dma_start(wq_sb[:], wq[:])
        for b in range(8):
            emit_in_dma("q", 0, b, nc.sync)
        nc.sync.dma_start(wk_sb[:], wk[:])
        nc.sync.dma_start(bq_sb[:], bq2[:])
        nc.sync.dma_start(bk_sb[:], bk2[:])
        for b in range(8):
            emit_in_dma("k", 0, b, nc.sync)
        nc.sync.dma_start(wv_sb[:], wv[:])
        nc.sync.dma_start(bv_sb[:], bvv[:])
        nc.sync.dma_start(wg_sb[:], wg[:])
        for b in range(8):
            emit_in_dma("v", 0, b, nc.sync)
        nc.sync.dma_start(id_sb[:], ident[:])
        nc.sync.dma_start(bg01_sb[:], bg01[:])
        nc.sync.dma_start(bg23_sb[:], bg23[:])
        nc.sync.dma_start(wfc0_sb[:], wfc[0:128, :])
        nc.sync.dma_start(wfc1_sb[:], wfc[128:256, :])

        # window 0 proj + G emitted directly (nothing else to overlap yet)
        for t in ("q", "k", "v"):
            cls, cell = make_proj_closures(t, 0)
            for c in cls:
                c()
            if t == "q":
                q0cell = cell
            if t == "v":
                for c in make_vpath_closures(0, cell):
                    c()
        make_g_closure(0, q0cell)()

        # ================= main pipeline =================
        push_window_feed(1)
        emit_window_attn(0)
        drain()

        push_window_feed(2)
        filler.extend(make_fc_closures(0))
        emit_window_attn(1)
        drain()

        push_window_feed(3)
        filler.extend(make_fc_closures(1))
        emit_window_attn(2)
        drain()

        filler.extend(make_fc_closures(2))
        emit_window_attn(3)
        drain()

        for c in make_fc_closures(3):
            c()


def shard_inputs(inputs):
    """full inputs -> list of 8 per-core in_maps (numpy, device layouts)"""
    f16 = np.float16
    f32 = np.float32
    q = np.asarray(inputs["q"], f32)[0]
    k = np.asarray(inputs["k"], f32)[0]
    v = np.asarray(inputs["v"], f32)[0]
    Wq = np.asarray(inputs["Wq"], f32)
    Wk = np.asarray(inputs["Wk"], f32)
    Wv = np.asarray(inputs["Wv"], f32)
    bq = np.asarray(inputs["bq"], f32)
    bk = np.asarray(inputs["bk"], f32)
    bv = np.asarray(inputs["bv"], f32)
    WG = np.asarray(inputs["WG"], f32)
    bG = np.asarray(inputs["bG"], f32)
    Wfc = np.asarray(inputs["Wfc"], f32)

    qT = np.ascontiguousarray(q.T.astype(f16))
    kT = np.ascontiguousarray(k.T.astype(f16))
    vT = np.ascontiguousarray(v.T.astype(f16))
    ident = np.eye(128, dtype=f32)

    def chunked(w):
        # [E, M] -> [128, 16*M]: e-chunk ec at cols [M*ec, M*ec+M)
        M = w.shape[1]
        return np.ascontiguousarray(
            w.reshape(16, 128, M).transpose(1, 0, 2).reshape(128, 16 * M))

    maps = []
    for h in range(HK):
        sl = slice(h * D, (h + 1) * D)
        wq_h = Wq[:, sl]
        wk_h = Wk[:, sl]
        wv_h = Wv[:, sl]
        m = {
            "qT": qT, "kT": kT, "vT": vT,
            "wq": chunked(np.concatenate([wq_h, wq_h], 1)).astype(f16),
            "wk": chunked(np.concatenate([wk_h, wk_h], 1)).astype(f16),
            "wv": chunked(wv_h).astype(f16),
            "bq2": np.concatenate([bq[sl], bq[sl]]).reshape(128, 1).copy(),
            "bk2": np.concatenate([bk[sl], bk[sl]]).reshape(128, 1).copy(),
            "bvv": bv[sl].reshape(64, 1).copy(),
            "wg": np.concatenate([WG[h], WG[h]], 0).copy(),  # [128, 256]
            "bg01": bG[h, 0:128].reshape(128, 1).copy(),
            "bg23": bG[h, 128:256].reshape(128, 1).copy(),
            "wfc": Wfc[h * 256:(h + 1) * 256, :].copy(),
            "ident": ident,
        }
        maps.append(m)
    return maps


_compiled = None
last_results = None


def get_compiled():
    global _compiled
    if _compiled is None:
        _compiled = build_program()
    return _compiled


def kernel(**inputs):
    global last_results
    nc = get_compiled()
    in_maps = shard_inputs(inputs)
    last_results = bass_utils.run_bass_kernel_spmd(
        nc, in_maps, core_ids=list(range(8)))
    bfc = np.asarray(inputs["bfc"], np.float32)
    acc = np.zeros((N, E), np.float64)
    for res in last_results.results:
        acc += res["out"].astype(np.float64)
    full = (acc + bfc[None, :].astype(np.float64)).astype(np.float32)
    return full.reshape(1, N, E)


# revision 11
# speedup vs baseline: 1.3771x; 1.0683x over previous
"""CompoundHeadAttention TRN2 kernel (v2 — software-pipelined schedule).

Full-input contract: kernel(**inputs) takes the unsharded tensors from
setup_inputs() and returns the full [1, 2048, 2048] float32 output.

Sharding (8 cores, tensor-parallel over the HK=8 kv heads):
  core h owns kv head h: its Wq/Wk/Wv column slice, its WG[h]/bG[h], and
  Wfc row-slice [h*256:(h+1)*256, :].  Each core computes its head's
  attention + its partial FC output [2048, 2048]; the host sums the 8
  partials and adds bfc (the "all-reduce" of the row-sharded FC).

v2 schedule (vs v1): the PE stream is kept dense — scores (ST) run two
chunks ahead of the PV consumer so the ACT exp latency never stalls the
in-order PE queue; projection/G/FC matmuls are interleaved into the
attention stream as filler at a per-window rate; softmax denominators
use reciprocal_approx_fast (0.7us vs 4us); input DMAs are batched
(2 e-chunks per instr) and issued from both the Sync and ACT queues.
"""

import os
import sys
from collections import deque

import numpy as np

if "/opt/trn_rl_repo" not in sys.path and os.path.isdir("/opt/trn_rl_repo"):
    sys.path.insert(0, "/opt/trn_rl_repo")

import concourse.bass as bass  # noqa: E402
import concourse.mybir as mybir  # noqa: E402
import concourse.tile as tile  # noqa: E402
from concourse import bacc  # noqa: E402
from concourse import bass_utils  # noqa: E402

F32 = mybir.dt.float32
F32R = mybir.dt.float32r
F16 = mybir.dt.float16
AF = mybir.ActivationFunctionType

N = 2048
E = 2048
HK = 8
D = 64
G = 4
NB = 4         # 512-wide n-windows
FILL_RATE = [6, 3, 2, 0]   # filler pops per attention chunk-slot, per window


def build_program():
    nc = bacc.Bacc("TRN2", target_bir_lowering=False, debug=False,
                   enable_asserts=False)

    qT = nc.dram_tensor("qT", [E, N], F16, kind="ExternalInput").ap()
    kT = nc.dram_tensor("kT", [E, N], F16, kind="ExternalInput").ap()
    vT = nc.dram_tensor("vT", [E, N], F16, kind="ExternalInput").ap()
    # weight chunk layout: [128, 16*M] — e-chunk ec occupies cols [M*ec, M*ec+M)
    wq = nc.dram_tensor("wq", [128, 16 * 128], F16, kind="ExternalInput").ap()
    wk = nc.dram_tensor("wk", [128, 16 * 128], F16, kind="ExternalInput").ap()
    wv = nc.dram_tensor("wv", [128, 16 * 64], F16, kind="ExternalInput").ap()
    bq2 = nc.dram_tensor("bq2", [128, 1], F32, kind="ExternalInput").ap()
    bk2 = nc.dram_tensor("bk2", [128, 1], F32, kind="ExternalInput").ap()
    bvv = nc.dram_tensor("bvv", [64, 1], F32, kind="ExternalInput").ap()
    wg = nc.dram_tensor("wg", [128, 256], F16, kind="ExternalInput").ap()
    bg01 = nc.dram_tensor("bg01", [128, 1], F32, kind="ExternalInput").ap()
    bg23 = nc.dram_tensor("bg23", [128, 1], F32, kind="ExternalInput").ap()
    wfc = nc.dram_tensor("wfc", [256, E], F16, kind="ExternalInput").ap()
    ident = nc.dram_tensor("ident", [128, 128], F32, kind="ExternalInput").ap()
    out = nc.dram_tensor("out", [N, E], F16, kind="ExternalOutput").ap()

    with tile.TileContext(nc) as tc:
        build_tile_kernel(tc, qT=qT, kT=kT, vT=vT, wq=wq, wk=wk, wv=wv,
                          bq2=bq2, bk2=bk2, bvv=bvv, wg=wg, bg01=bg01,
                          bg23=bg23, wfc=wfc, ident=ident, out=out)
    nc.compile()
    return nc


def build_tile_kernel(tc, *, qT, kT, vT, wq, wk, wv, bq2, bk2, bvv, wg,
                      bg01, bg23, wfc, ident, out):
    nc = tc.nc

    import contextlib
    ctx = contextlib.ExitStack()
    ctx.__enter__()
    cp = ctx.enter_context(tc.tile_pool(name="persist", bufs=1))

    def ptile(shape, dtype, name):
        return cp.tile(shape, dtype, tag=name, name=name)

    # ---- persistent constants / state in SBUF ----
    wq_sb = ptile([128, 16 * 128], F16, "wq_sb")
    wk_sb = ptile([128, 16 * 128], F16, "wk_sb")
    wv_sb = ptile([128, 16 * 64], F16, "wv_sb")
    wg_sb = ptile([128, 256], F16, "wg_sb")
    wfc0_sb = ptile([128, E], F16, "wfc0_sb")
    wfc1_sb = ptile([128, E], F16, "wfc1_sb")
    id_sb = ptile([128, 128], F32, "id_sb")
    bq_sb = ptile([128, 1], F32, "bq_sb")
    bk_sb = ptile([128, 1], F32, "bk_sb")
    bv_sb = ptile([64, 1], F32, "bv_sb")
    bg01_sb = ptile([128, 1], F32, "bg01_sb")
    bg23_sb = ptile([128, 1], F32, "bg23_sb")
    ones_sb = ptile([128, 1], F32, "ones_sb")
    warm_sb = ptile([1, 1], F32, "warm_sb")

    kt_w = [ptile([128, 512], F16, f"kt{j}") for j in range(NB)]
    vo_w = [ptile([128, 4 * 65], F32R, f"vo{j}") for j in range(NB)]

    with ctx:
        in_pool = ctx.enter_context(tc.tile_pool(name="in_pool", bufs=8))
        qt_pool = ctx.enter_context(tc.tile_pool(name="qt_pool", bufs=2))
        qg_pool = ctx.enter_context(tc.tile_pool(name="qg_pool", bufs=2))
        hid_pool = ctx.enter_context(tc.tile_pool(name="hid_pool", bufs=2))
        vt_pool = ctx.enter_context(tc.tile_pool(name="vt_pool", bufs=2))
        pt_pool = ctx.enter_context(tc.tile_pool(name="pt_pool", bufs=4))
        rec_pool = ctx.enter_context(tc.tile_pool(name="rec_pool", bufs=2))
        fco_pool = ctx.enter_context(tc.tile_pool(name="fco_pool", bufs=2))
        misc_ps = ctx.enter_context(
            tc.tile_pool(name="misc_ps", bufs=2, space="PSUM"))
        st_ps = ctx.enter_context(
            tc.tile_pool(name="st_ps", bufs=2, space="PSUM"))
        pv_ps = ctx.enter_context(
            tc.tile_pool(name="pv_ps", bufs=2, space="PSUM"))

        # ---------- shared state set as emission progresses ----------
        in_tiles = {}    # (tensor, pair, batch) -> sbuf tile [128, 2048]
        qg01_w = [None] * NB
        qg23_w = [None] * NB
        hid01_w = [None] * NB
        hid23_w = [None] * NB

        filler = deque()

        def fill(n):
            c = 0
            while filler and c < n:
                filler.popleft()()
                c += 1

        def drain():
            while filler:
                filler.popleft()()

        # ---------- DMA emission helpers ----------
        TSRC = {"q": (qT, "qin"), "k": (kT, "kin"), "v": (vT, "vin")}

        def emit_in_dma(t, P, b, eng):
            src_t, tag = TSRC[t]
            ti = in_pool.tile([128, 2048], F16, tag=tag, name=f"{t}in")
            src = src_t[bass.ds(256 * b, 256), bass.ds(P * 1024, 1024)]
            eng.dma_start(ti[:].rearrange("p (c n) -> p c n", c=2),
                          src.rearrange("(c p) n -> p c n", p=128))
            in_tiles[(t, P, b)] = ti

        # ---------- projection emission (per tensor, per window) ----------
        def make_proj_closures(t, j):
            """16 matmuls (8 batch-closures) + 1 bias closure for tensor t,
            window j. Sets qt/kt/vt state."""
            P, h = j // 2, j % 2
            cell = {}

            def mk_mm(b):
                def go():
                    if b == 0:
                        rows = 64 if t == "v" else 128
                        cell["ps"] = misc_ps.tile([rows, 512], F32, tag="mm",
                                                  name=f"{t}_ps")
                    w_sb = {"q": wq_sb, "k": wk_sb, "v": wv_sb}[t]
                    M = 64 if t == "v" else 128
                    ps = cell["ps"]
                    for c in range(2):
                        ec = 2 * b + c
                        mv = in_tiles[(t, P, b)][
                            :, 1024 * c + 512 * h: 1024 * c + 512 * h + 512]
                        nc.tensor.matmul(ps[:], w_sb[:, bass.ts(ec, M)], mv,
                                         start=(ec == 0), stop=(ec == 15))
                return go

            def bias():
                ps = cell["ps"]
                if t == "q":
                    qt = qt_pool.tile([128, 512], F16, tag="qt", name="qt")
                    nc.scalar.activation(qt[:], ps[:], AF.Identity,
                                         bias=bq_sb[:])
                    cell["qt"] = qt
                elif t == "k":
                    nc.scalar.activation(kt_w[j][:], ps[:], AF.Identity,
                                         bias=bk_sb[:])
                else:
                    vt = vt_pool.tile([64, 512], F32, tag="vt", name="vt")
                    nc.scalar.activation(vt[:], ps[:], AF.Identity,
                                         bias=bv_sb[:])
                    cell["vt"] = vt

            return [mk_mm(b) for b in range(8)] + [bias], cell

        def make_vpath_closures(j, vcell):
            """PE transposes + DVE copies: VT -> vo_w[j] data columns."""
            tr_cell = {}

            def tr():
                tr_ps = misc_ps.tile([128, 256], F32, tag="mm", name="tr_ps")
                for t4 in range(4):
                    nc.tensor.transpose(tr_ps[:, bass.ts(t4, 64)],
                                        vcell["vt"][:, bass.ts(t4, 128)],
                                        id_sb[0:64, 0:64])
                tr_cell["tr"] = tr_ps

            def cp_out():
                vo3 = vo_w[j][:].rearrange("p (t c) -> p t c", c=65)
                for t4 in range(4):
                    nc.vector.tensor_copy(vo3[:, t4, 0:64],
                                          tr_cell["tr"][:, bass.ts(t4, 64)])

            return [tr, cp_out]

        def make_g_closure(j, qcell):
            def go():
                g01 = misc_ps.tile([128, 512], F32, tag="mm", name="g01_ps")
                nc.tensor.matmul(g01[:], wg_sb[0:64, 0:128],
                                 qcell["qt"][0:64, :], start=True, stop=True)
                g23 = misc_ps.tile([128, 512], F32, tag="mm", name="g23_ps")
                nc.tensor.matmul(g23[:], wg_sb[64:128, 128:256],
                                 qcell["qt"][64:128, :], start=True, stop=True)
                qg01 = qg_pool.tile([128, 512], F16, tag="qg01", name="qg01")
                qg23 = qg_pool.tile([128, 512], F16, tag="qg23", name="qg23")
                nc.scalar.activation(qg01[:], g01[:], AF.Identity,
                                     bias=bg01_sb[:])
                nc.scalar.activation(qg23[:], g23[:], AF.Identity,
                                     bias=bg23_sb[:])
                qg01_w[j] = qg01
                qg23_w[j] = qg23
            return go

        def push_window_feed(j):
            """Queue proj+G for window j as filler closures.  For j==1,
            interleave the pair-1 input DMAs right behind the proj closure
            that frees each input buffer."""
            def extend_interleaved(t, cls):
                mms, bias = cls[:8], cls[8]
                for b, mm in enumerate(mms):
                    filler.append(mm)
                    if j == 1:
                        filler.append(
                            lambda t=t, b=b: emit_in_dma(t, 1, b, nc.sync))
                filler.append(bias)

            qcl, qcell = make_proj_closures("q", j)
            extend_interleaved("q", qcl)
            kcl, _ = make_proj_closures("k", j)
            extend_interleaved("k", kcl)
            vcl, vcell = make_proj_closures("v", j)
            extend_interleaved("v", vcl)
            filler.extend(make_vpath_closures(j, vcell))
            filler.append(make_g_closure(j, qcell))

        # ---------- FC emission ----------
        def make_fc_closures(j):
            cls = []
            for m in range(4):
                cell = {}
                for eo in range(4):
                    def go(m=m, eo=eo, cell=cell):
                        if eo == 0:
                            cell["stage"] = fco_pool.tile(
                                [128, 2048], F16, tag="stage", name="stage")
                        fc_ps = misc_ps.tile([128, 512], F32, tag="mm",
                                             name="fc_ps")
                        nc.tensor.matmul(fc_ps[:],
                                         hid01_w[j][:, bass.ts(m, 128)],
                                         wfc0_sb[:, bass.ts(eo, 512)],
                                         start=True, stop=False)
                        nc.tensor.matmul(fc_ps[:],
                                         hid23_w[j][:, bass.ts(m, 128)],
                                         wfc1_sb[:, bass.ts(eo, 512)],
                                         start=False, stop=True)
                        nc.vector.tensor_copy(
                            cell["stage"][:, bass.ts(eo, 512)], fc_ps[:])
                        if eo == 3:
                            nc.sync.dma_start(
                                out[512 * j + 128 * m: 512 * j + 128 * m + 128,
                                    :],
                                cell["stage"][:])
                    cls.append(go)
            return cls

        # ---------- attention emission ----------
        def emit_window_attn(j):
            K = 4 * j + 4
            for pair, qg_of in ((0, qg01_w), (1, qg23_w)):
                qg = qg_of[j]
                pv_a = pv_ps.tile([65, 512], F32, tag="pv", name="pv_a")
                pv_b = pv_ps.tile([65, 512], F32, tag="pv", name="pv_b")
                pts = {}

                def st_step(k):
                    kt_c = kt_w[k // 4]
                    ks = bass.ts(k % 4, 128)
                    i = k - 4 * j
                    off = max(0, 128 * i)
                    if off == 384:
                        off = 256
                    st = st_ps.tile([128, 1024], F32, tag="st", name="st")
                    nc.tensor.matmul(st[:, off:512], kt_c[0:64, ks],
                                     qg[0:64, off:512], start=True, stop=True)
                    nc.tensor.matmul(st[:, 512 + off:1024], kt_c[64:128, ks],
                                     qg[64:128, off:512],
                                     start=True, stop=True)
                    pt = pt_pool.tile([128, 1024], F32R, tag="pt", name="pt")
                    st3 = st[:].rearrange("p (g c) -> p g c", c=512)
                    pt3 = pt[:].rearrange("p (g c) -> p g c", c=512)
                    nc.scalar.activation(pt3[:, :, off:512],
                                         st3[:, :, off:512],
                                         AF.Exp, scale=8.0)
                    if i >= 0:
                        mw = 128 * i + 128 - off
                        nc.gpsimd.affine_select(
                            out=pt3[:, :, off:off + mw],
                            in_=pt3[:, :, off:off + mw],
                            compare_op=mybir.AluOpType.is_ge,
                            fill=0.0, base=-(128 * i - off),
                            pattern=[[0, 2], [1, mw]],
                            channel_multiplier=-1)
                    pts[k] = (pt, off)

                def pv_step(k):
                    pt, off = pts.pop(k)
                    vo_c = vo_w[k // 4]
                    vsl = vo_c[:, (k % 4) * 65:(k % 4) * 65 + 65]
                    nc.tensor.matmul(pv_a[:, off:512], vsl, pt[:, off:512],
                                     start=(k == 0), stop=(k == K - 1))
                    nc.tensor.matmul(pv_b[:, off:512], vsl,
                                     pt[:, 512 + off:1024],
                                     start=(k == 0), stop=(k == K - 1))

                st_step(0)
                if K > 1:
                    st_step(1)
                for k in range(K):
                    if k + 2 < K:
                        st_step(k + 2)
                    # extra filler at the pair start covers the previous
                    # pair's normalize chain before pv psum reuse
                    fill(FILL_RATE[j] + ((4 if j == 3 else 2) if k < 2 else 0))
                    pv_step(k)

                # normalize: hid[g-half] = pv[0:64] * (1/pv[64])
                if pair == 0:
                    hid = hid_pool.tile([128, 512], F16, tag="hid01",
                                        name="hid01")
                    hid01_w[j] = hid
                else:
                    hid = hid_pool.tile([128, 512], F16, tag="hid23",
                                        name="hid23")
                    hid23_w[j] = hid
                den_a = rec_pool.tile([1, 512], F32, tag="den", name="den_a")
                nc.vector.tensor_copy(den_a[:], pv_a[64:65, :])
                rec_a = rec_pool.tile([1, 512], F32, tag="rec", name="rec_a")
                nc.vector.reciprocal_approx_fast(rec_a[:], den_a[:])
                recr_a = rec_pool.tile([64, 512], F32, tag="recr",
                                       name="recr_a")
                nc.gpsimd.partition_broadcast(recr_a[:], rec_a[:])
                den_b = rec_pool.tile([1, 512], F32, tag="den", name="den_b")
                nc.vector.tensor_copy(den_b[:], pv_b[64:65, :])
                rec_b = rec_pool.tile([1, 512], F32, tag="rec", name="rec_b")
                nc.vector.reciprocal_approx_fast(rec_b[:], den_b[:])
                recr_b = rec_pool.tile([64, 512], F32, tag="recr",
                                       name="recr_b")
                nc.gpsimd.partition_broadcast(recr_b[:], rec_b[:])
                nc.vector.tensor_mul(hid[0:64, :], pv_a[0:64, :], recr_a[:])
                nc.vector.tensor_mul(hid[64:128, :], pv_b[0:64, :],
                                     recr_b[:])

        # ================= prologue =================
        nc.vector.memset(ones_sb[:], 1.0)
        nc.scalar.activation(warm_sb[:], ones_sb[0:1, :], AF.Exp, scale=1.0)
        for j in range(NB):
            for t4 in range(4):
                nc.vector.tensor_copy(
                    vo_w[j][:, t4 * 65 + 64: t4 * 65 + 65], ones_sb[:])

        nc.sync.dma_start(wq_sb[:], wq[:])
        for b in range(8):
            emit_in_dma("q", 0, b, nc.sync)
        nc.sync.dma_start(wk_sb[:], wk[:])
        nc.sync.dma_start(bq_sb[:], bq2[:])
        nc.sync.dma_start(bk_sb[:], bk2[:])
        for b in range(8):
            emit_in_dma("k", 0, b, nc.scalar)
        nc.sync.dma_start(wv_sb[:], wv[:])
        nc.sync.dma_start(bv_sb[:], bvv[:])
        nc.sync.dma_start(wg_sb[:], wg[:])
        for b in range(8):
            emit_in_dma("v", 0, b, nc.scalar)
        nc.sync.dma_start(id_sb[:], ident[:])
        nc.sync.dma_start(bg01_sb[:], bg01[:])
        nc.sync.dma_start(bg23_sb[:], bg23[:])
        nc.sync.dma_start(wfc0_sb[:], wfc[0:128, :])
        nc.sync.dma_start(wfc1_sb[:], wfc[128:256, :])

        # window 0 proj + G emitted directly (nothing else to overlap yet)
        for t in ("q", "k", "v"):
            cls, cell = make_proj_closures(t, 0)
            for c in cls:
                c()
            if t == "q":
                q0cell = cell
            if t == "v":
                for c in make_vpath_closures(0, cell):
                    c()
        make_g_closure(0, q0cell)()

        # ================= main pipeline =================
        push_window_feed(1)
        emit_window_attn(0)
        drain()

        push_window_feed(2)
        filler.extend(make_fc_closures(0))
        emit_window_attn(1)
        drain()

        push_window_feed(3)
        filler.extend(make_fc_closures(1))
        emit_window_attn(2)
        drain()

        filler.extend(make_fc_closures(2))
        emit_window_attn(3)
        drain()

        for c in make_fc_closures(3):
            c()


def shard_inputs(inputs):
    """full inputs -> list of 8 per-core in_maps (numpy, device layouts)"""
    f16 = np.float16
    f32 = np.float32
    q = np.asarray(inputs["q"], f32)[0]
    k = np.asarray(inputs["k"], f32)[0]
    v = np.asarray(inputs["v"], f32)[0]
    Wq = np.asarray(inputs["Wq"], f32)
    Wk = np.asarray(inputs["Wk"], f32)
    Wv = np.asarray(inputs["Wv"], f32)
    bq = np.asarray(inputs["bq"], f32)
    bk = np.asarray(inputs["bk"], f32)
    bv = np.asarray(inputs["bv"], f32)
    WG = np.asarray(inputs["WG"], f32)
    bG = np.asarray(inputs["bG"], f32)
    Wfc = np.asarray(inputs["Wfc"], f32)

    qT = np.ascontiguousarray(q.T.astype(f16))
    kT = np.ascontiguousarray(k.T.astype(f16))
    vT = np.ascontiguousarray(v.T.astype(f16))
    ident = np.eye(128, dtype=f32)

    def chunked(w):
        # [E, M] -> [128, 16*M]: e-chunk ec at cols [M*ec, M*ec+M)
        M = w.shape[1]
        return np.ascontiguousarray(
            w.reshape(16, 128, M).transpose(1, 0, 2).reshape(128, 16 * M))

    maps = []
    for h in range(HK):
        sl = slice(h * D, (h + 1) * D)
        wq_h = Wq[:, sl]
        wk_h = Wk[:, sl]
        wv_h = Wv[:, sl]
        m = {
            "qT": qT, "kT": kT, "vT": vT,
            "wq": chunked(np.concatenate([wq_h, wq_h], 1)).astype(f16),
            "wk": chunked(np.concatenate([wk_h, wk_h], 1)).astype(f16),
            "wv": chunked(wv_h).astype(f16),
            "bq2": np.concatenate([bq[sl], bq[sl]]).reshape(128, 1).copy(),
            "bk2": np.concatenate([bk[sl], bk[sl]]).reshape(128, 1).copy(),
            "bvv": bv[sl].reshape(64, 1).copy(),
            "wg": np.concatenate([WG[h], WG[h]], 0).astype(f16),  # [128, 256]
            "bg01": bG[h, 0:128].reshape(128, 1).copy(),
            "bg23": bG[h, 128:256].reshape(128, 1).copy(),
            "wfc": Wfc[h * 256:(h + 1) * 256, :].astype(f16),
            "ident": ident,
        }
        maps.append(m)
    return maps


_compiled = None
last_results = None


def get_compiled():
    global _compiled
    if _compiled is None:
        _compiled = build_program()
    return _compiled


def kernel(**inputs):
    global last_results
    nc = get_compiled()
    in_maps = shard_inputs(inputs)
    last_results = bass_utils.run_bass_kernel_spmd(
        nc, in_maps, core_ids=list(range(8)))
    bfc = np.asarray(inputs["bfc"], np.float32)
    acc = np.zeros((N, E), np.float64)
    for res in last_results.results:
        acc += res["out"].astype(np.float64)
    full = (acc + bfc[None, :].astype(np.float64)).astype(np.float32)
    return full.reshape(1, N, E)


# revision 13
# speedup vs baseline: 1.5044x; 1.0925x over previous
"""CompoundHeadAttention TRN2 kernel (v2 — software-pipelined schedule).

Full-input contract: kernel(**inputs) takes the unsharded tensors from
setup_inputs() and returns the full [1, 2048, 2048] float32 output.

Sharding (8 cores, tensor-parallel over the HK=8 kv heads):
  core h owns kv head h: its Wq/Wk/Wv column slice, its WG[h]/bG[h], and
  Wfc row-slice [h*256:(h+1)*256, :].  Each core computes its head's
  attention + its partial FC output [2048, 2048]; the host sums the 8
  partials and adds bfc (the "all-reduce" of the row-sharded FC).

v2 schedule (vs v1): the PE stream is kept dense — scores (ST) run two
chunks ahead of the PV consumer so the ACT exp latency never stalls the
in-order PE queue; projection/G/FC matmuls are interleaved into the
attention stream as filler at a per-window rate; softmax denominators
use reciprocal_approx_fast (0.7us vs 4us); input DMAs are batched
(2 e-chunks per instr) and issued from both the Sync and ACT queues.
"""

import os
import sys
from collections import deque

import numpy as np

if "/opt/trn_rl_repo" not in sys.path and os.path.isdir("/opt/trn_rl_repo"):
    sys.path.insert(0, "/opt/trn_rl_repo")

import concourse.bass as bass  # noqa: E402
import concourse.mybir as mybir  # noqa: E402
import concourse.tile as tile  # noqa: E402
from concourse import bacc  # noqa: E402
from concourse import bass_utils  # noqa: E402

F32 = mybir.dt.float32
F32R = mybir.dt.float32r
F16 = mybir.dt.float16
AF = mybir.ActivationFunctionType

N = 2048
E = 2048
HK = 8
D = 64
G = 4
NB = 4         # 512-wide n-windows
FILL_RATE = [6, 3, 2, 0]   # filler pops per attention chunk-slot, per window


def build_program():
    nc = bacc.Bacc("TRN2", target_bir_lowering=False, debug=False,
                   enable_asserts=False)

    qT = nc.dram_tensor("qT", [E, N], F16, kind="ExternalInput").ap()
    kT = nc.dram_tensor("kT", [E, N], F16, kind="ExternalInput").ap()
    vT = nc.dram_tensor("vT", [E, N], F16, kind="ExternalInput").ap()
    # weight chunk layout: [128, 16*M] — e-chunk ec occupies cols [M*ec, M*ec+M)
    wq = nc.dram_tensor("wq", [128, 16 * 128], F16, kind="ExternalInput").ap()
    wk = nc.dram_tensor("wk", [128, 16 * 128], F16, kind="ExternalInput").ap()
    wv = nc.dram_tensor("wv", [128, 16 * 64], F16, kind="ExternalInput").ap()
    bq2 = nc.dram_tensor("bq2", [128, 1], F32, kind="ExternalInput").ap()
    bk2 = nc.dram_tensor("bk2", [128, 1], F32, kind="ExternalInput").ap()
    bvv = nc.dram_tensor("bvv", [64, 1], F32, kind="ExternalInput").ap()
    wg = nc.dram_tensor("wg", [128, 256], F16, kind="ExternalInput").ap()
    bg01 = nc.dram_tensor("bg01", [128, 1], F32, kind="ExternalInput").ap()
    bg23 = nc.dram_tensor("bg23", [128, 1], F32, kind="ExternalInput").ap()
    wfc = nc.dram_tensor("wfc", [256, E], F16, kind="ExternalInput").ap()
    ident = nc.dram_tensor("ident", [128, 128], F32, kind="ExternalInput").ap()
    out = nc.dram_tensor("out", [N, E], F16, kind="ExternalOutput").ap()

    with tile.TileContext(nc) as tc:
        build_tile_kernel(tc, qT=qT, kT=kT, vT=vT, wq=wq, wk=wk, wv=wv,
                          bq2=bq2, bk2=bk2, bvv=bvv, wg=wg, bg01=bg01,
                          bg23=bg23, wfc=wfc, ident=ident, out=out)
    nc.compile()
    return nc


def build_tile_kernel(tc, *, qT, kT, vT, wq, wk, wv, bq2, bk2, bvv, wg,
                      bg01, bg23, wfc, ident, out):
    nc = tc.nc

    import contextlib
    ctx = contextlib.ExitStack()
    ctx.__enter__()
    cp = ctx.enter_context(tc.tile_pool(name="persist", bufs=1))

    def ptile(shape, dtype, name):
        return cp.tile(shape, dtype, tag=name, name=name)

    # ---- persistent constants / state in SBUF ----
    wq_sb = ptile([128, 16 * 128], F16, "wq_sb")
    wk_sb = ptile([128, 16 * 128], F16, "wk_sb")
    wv_sb = ptile([128, 16 * 64], F16, "wv_sb")
    wg_sb = ptile([128, 256], F16, "wg_sb")
    wfc0_sb = ptile([128, E], F16, "wfc0_sb")
    wfc1_sb = ptile([128, E], F16, "wfc1_sb")
    id_sb = ptile([128, 128], F32, "id_sb")
    bq_sb = ptile([128, 1], F32, "bq_sb")
    bk_sb = ptile([128, 1], F32, "bk_sb")
    bv_sb = ptile([64, 1], F32, "bv_sb")
    bg01_sb = ptile([128, 1], F32, "bg01_sb")
    bg23_sb = ptile([128, 1], F32, "bg23_sb")
    ones_sb = ptile([128, 1], F32, "ones_sb")
    warm_sb = ptile([1, 1], F32, "warm_sb")

    kt_w = [ptile([128, 512], F16, f"kt{j}") for j in range(NB)]
    vo_w = [ptile([128, 4 * 65], F32R, f"vo{j}") for j in range(NB)]

    with ctx:
        in_pool = ctx.enter_context(tc.tile_pool(name="in_pool", bufs=10))
        qt_pool = ctx.enter_context(tc.tile_pool(name="qt_pool", bufs=2))
        qg_pool = ctx.enter_context(tc.tile_pool(name="qg_pool", bufs=2))
        hid_pool = ctx.enter_context(tc.tile_pool(name="hid_pool", bufs=2))
        vt_pool = ctx.enter_context(tc.tile_pool(name="vt_pool", bufs=2))
        pt_pool = ctx.enter_context(tc.tile_pool(name="pt_pool", bufs=4))
        rec_pool = ctx.enter_context(tc.tile_pool(name="rec_pool", bufs=2))
        fco_pool = ctx.enter_context(tc.tile_pool(name="fco_pool", bufs=2))
        misc_ps = ctx.enter_context(
            tc.tile_pool(name="misc_ps", bufs=2, space="PSUM"))
        st_ps = ctx.enter_context(
            tc.tile_pool(name="st_ps", bufs=2, space="PSUM"))
        pv_ps = ctx.enter_context(
            tc.tile_pool(name="pv_ps", bufs=2, space="PSUM"))

        # ---------- shared state set as emission progresses ----------
        in_tiles = {}    # (tensor, pair, batch) -> sbuf tile [128, 2048]
        qg01_w = [None] * NB
        qg23_w = [None] * NB
        hid01_w = [None] * NB
        hid23_w = [None] * NB

        filler = deque()

        def fill(n):
            c = 0
            while filler and c < n:
                filler.popleft()()
                c += 1

        def drain():
            while filler:
                filler.popleft()()

        # ---------- DMA emission helpers ----------
        TSRC = {"q": (qT, "qin"), "k": (kT, "kin"), "v": (vT, "vin")}

        def emit_in_dma(t, P, b, eng):
            src_t, tag = TSRC[t]
            ti = in_pool.tile([128, 2048], F16, tag=tag, name=f"{t}in")
            src = src_t[bass.ds(256 * b, 256), bass.ds(P * 1024, 1024)]
            eng.dma_start(ti[:].rearrange("p (c n) -> p c n", c=2),
                          src.rearrange("(c p) n -> p c n", p=128))
            in_tiles[(t, P, b)] = ti

        # ---------- projection emission (per tensor, per window) ----------
        def make_proj_closures(t, j):
            """16 matmuls (8 batch-closures) + 1 bias closure for tensor t,
            window j. Sets qt/kt/vt state."""
            P, h = j // 2, j % 2
            cell = {}

            def mk_mm(b):
                def go():
                    if b == 0:
                        rows = 64 if t == "v" else 128
                        cell["ps"] = misc_ps.tile([rows, 512], F32, tag="mm",
                                                  name=f"{t}_ps")
                    w_sb = {"q": wq_sb, "k": wk_sb, "v": wv_sb}[t]
                    M = 64 if t == "v" else 128
                    ps = cell["ps"]
                    for c in range(2):
                        ec = 2 * b + c
                        mv = in_tiles[(t, P, b)][
                            :, 1024 * c + 512 * h: 1024 * c + 512 * h + 512]
                        nc.tensor.matmul(ps[:], w_sb[:, bass.ts(ec, M)], mv,
                                         start=(ec == 0), stop=(ec == 15))
                return go

            def bias():
                ps = cell["ps"]
                if t == "q":
                    qt = qt_pool.tile([128, 512], F16, tag="qt", name="qt")
                    nc.scalar.activation(qt[:], ps[:], AF.Identity,
                                         bias=bq_sb[:])
                    cell["qt"] = qt
                elif t == "k":
                    nc.scalar.activation(kt_w[j][:], ps[:], AF.Identity,
                                         bias=bk_sb[:])
                else:
                    vt = vt_pool.tile([64, 512], F32, tag="vt", name="vt")
                    nc.scalar.activation(vt[:], ps[:], AF.Identity,
                                         bias=bv_sb[:])
                    cell["vt"] = vt

            return [mk_mm(b) for b in range(8)] + [bias], cell

        def make_vpath_closures(j, vcell):
            """PE transposes + DVE copies: VT -> vo_w[j] data columns."""
            tr_cell = {}

            def tr():
                tr_ps = misc_ps.tile([128, 256], F32, tag="mm", name="tr_ps")
                for t4 in range(4):
                    nc.tensor.transpose(tr_ps[:, bass.ts(t4, 64)],
                                        vcell["vt"][:, bass.ts(t4, 128)],
                                        id_sb[0:64, 0:64])
                tr_cell["tr"] = tr_ps

            def cp_out():
                vo3 = vo_w[j][:].rearrange("p (t c) -> p t c", c=65)
                for t4 in range(4):
                    nc.vector.tensor_copy(vo3[:, t4, 0:64],
                                          tr_cell["tr"][:, bass.ts(t4, 64)])

            return [tr, cp_out]

        def make_g_closure(j, qcell):
            def go():
                g01 = misc_ps.tile([128, 512], F32, tag="mm", name="g01_ps")
                nc.tensor.matmul(g01[:], wg_sb[0:64, 0:128],
                                 qcell["qt"][0:64, :], start=True, stop=True)
                g23 = misc_ps.tile([128, 512], F32, tag="mm", name="g23_ps")
                nc.tensor.matmul(g23[:], wg_sb[64:128, 128:256],
                                 qcell["qt"][64:128, :], start=True, stop=True)
                qg01 = qg_pool.tile([128, 512], F16, tag="qg01", name="qg01")
                qg23 = qg_pool.tile([128, 512], F16, tag="qg23", name="qg23")
                nc.scalar.activation(qg01[:], g01[:], AF.Identity,
                                     bias=bg01_sb[:])
                nc.scalar.activation(qg23[:], g23[:], AF.Identity,
                                     bias=bg23_sb[:])
                qg01_w[j] = qg01
                qg23_w[j] = qg23
            return go

        def push_window_feed(j):
            """Queue proj+G for window j as filler closures.  For j==1,
            interleave the pair-1 input DMAs right behind the proj closure
            that frees each input buffer."""
            def extend_interleaved(t, cls):
                mms, bias = cls[:8], cls[8]
                for b, mm in enumerate(mms):
                    filler.append(mm)
                    if j == 1 and b >= 2:
                        filler.append(
                            lambda t=t, b=b: emit_in_dma(t, 1, b, nc.sync))
                filler.append(bias)

            qcl, qcell = make_proj_closures("q", j)
            extend_interleaved("q", qcl)
            kcl, _ = make_proj_closures("k", j)
            extend_interleaved("k", kcl)
            vcl, vcell = make_proj_closures("v", j)
            extend_interleaved("v", vcl)
            filler.extend(make_vpath_closures(j, vcell))
            filler.append(make_g_closure(j, qcell))

        # ---------- FC emission ----------
        def make_fc_closures(j):
            cls = []
            for m in range(4):
                cell = {}
                for eo in range(4):
                    def go(m=m, eo=eo, cell=cell):
                        if eo == 0:
                            cell["stage"] = fco_pool.tile(
                                [128, 2048], F16, tag="stage", name="stage")
                        fc_ps = misc_ps.tile([128, 512], F32, tag="mm",
                                             name="fc_ps")
                        nc.tensor.matmul(fc_ps[:],
                                         hid01_w[j][:, bass.ts(m, 128)],
                                         wfc0_sb[:, bass.ts(eo, 512)],
                                         start=True, stop=False)
                        nc.tensor.matmul(fc_ps[:],
                                         hid23_w[j][:, bass.ts(m, 128)],
                                         wfc1_sb[:, bass.ts(eo, 512)],
                                         start=False, stop=True)
                        nc.vector.tensor_copy(
                            cell["stage"][:, bass.ts(eo, 512)], fc_ps[:])
                        if eo == 3:
                            nc.sync.dma_start(
                                out[512 * j + 128 * m: 512 * j + 128 * m + 128,
                                    :],
                                cell["stage"][:])
                    cls.append(go)
            return cls

        # ---------- attention emission ----------
        def emit_window_attn(j):
            K = 4 * j + 4
            for pair, qg_of in ((0, qg01_w), (1, qg23_w)):
                qg = qg_of[j]
                pv_a = pv_ps.tile([65, 512], F32, tag="pv", name="pv_a")
                pv_b = pv_ps.tile([65, 512], F32, tag="pv", name="pv_b")
                pts = {}

                def st_step(k):
                    kt_c = kt_w[k // 4]
                    ks = bass.ts(k % 4, 128)
                    i = k - 4 * j
                    off = max(0, 128 * i)
                    if off == 384:
                        off = 256
                    st = st_ps.tile([128, 1024], F32, tag="st", name="st")
                    nc.tensor.matmul(st[:, off:512], kt_c[0:64, ks],
                                     qg[0:64, off:512], start=True, stop=True)
                    nc.tensor.matmul(st[:, 512 + off:1024], kt_c[64:128, ks],
                                     qg[64:128, off:512],
                                     start=True, stop=True)
                    pt = pt_pool.tile([128, 1024], F32R, tag="pt", name="pt")
                    st3 = st[:].rearrange("p (g c) -> p g c", c=512)
                    pt3 = pt[:].rearrange("p (g c) -> p g c", c=512)
                    nc.scalar.activation(pt3[:, :, off:512],
                                         st3[:, :, off:512],
                                         AF.Exp, scale=8.0)
                    if i >= 0:
                        mw = 128 * i + 128 - off
                        nc.gpsimd.affine_select(
                            out=pt3[:, :, off:off + mw],
                            in_=pt3[:, :, off:off + mw],
                            compare_op=mybir.AluOpType.is_ge,
                            fill=0.0, base=-(128 * i - off),
                            pattern=[[0, 2], [1, mw]],
                            channel_multiplier=-1)
                    pts[k] = (pt, off)

                def pv_step(k):
                    pt, off = pts.pop(k)
                    vo_c = vo_w[k // 4]
                    vsl = vo_c[:, (k % 4) * 65:(k % 4) * 65 + 65]
                    nc.tensor.matmul(pv_a[:, off:512], vsl, pt[:, off:512],
                                     start=(k == 0), stop=(k == K - 1))
                    nc.tensor.matmul(pv_b[:, off:512], vsl,
                                     pt[:, 512 + off:1024],
                                     start=(k == 0), stop=(k == K - 1))

                fill(2)
                st_step(0)
                if K > 1:
                    st_step(1)
                for k in range(K):
                    if k + 2 < K:
                        st_step(k + 2)
                    # extra filler at the pair start covers the previous
                    # pair's normalize chain before pv psum reuse
                    fill(FILL_RATE[j] + ((3 if j == 3 else 2) if k < 2 else 0))
                    pv_step(k)

                # normalize: hid[g-half] = pv[0:64] * (1/pv[64])
                if pair == 0:
                    hid = hid_pool.tile([128, 512], F16, tag="hid01",
                                        name="hid01")
                    hid01_w[j] = hid
                else:
                    hid = hid_pool.tile([128, 512], F16, tag="hid23",
                                        name="hid23")
                    hid23_w[j] = hid
                den_a = rec_pool.tile([1, 512], F32, tag="den", name="den_a")
                nc.vector.tensor_copy(den_a[:], pv_a[64:65, :])
                rec_a = rec_pool.tile([1, 512], F32, tag="rec", name="rec_a")
                nc.vector.reciprocal_approx_fast(rec_a[:], den_a[:])
                recr_a = rec_pool.tile([64, 512], F32, tag="recr",
                                       name="recr_a")
                nc.gpsimd.partition_broadcast(recr_a[:], rec_a[:])
                den_b = rec_pool.tile([1, 512], F32, tag="den", name="den_b")
                nc.vector.tensor_copy(den_b[:], pv_b[64:65, :])
                rec_b = rec_pool.tile([1, 512], F32, tag="rec", name="rec_b")
                nc.vector.reciprocal_approx_fast(rec_b[:], den_b[:])
                recr_b = rec_pool.tile([64, 512], F32, tag="recr",
                                       name="recr_b")
                nc.gpsimd.partition_broadcast(recr_b[:], rec_b[:])
                nc.vector.tensor_mul(hid[0:64, :], pv_a[0:64, :], recr_a[:])
                nc.vector.tensor_mul(hid[64:128, :], pv_b[0:64, :],
                                     recr_b[:])

        # ================= prologue =================
        nc.vector.memset(ones_sb[:], 1.0)
        nc.scalar.activation(warm_sb[:], ones_sb[0:1, :], AF.Exp, scale=1.0)
        for j in range(NB):
            for t4 in range(4):
                nc.vector.tensor_copy(
                    vo_w[j][:, t4 * 65 + 64: t4 * 65 + 65], ones_sb[:])

        nc.sync.dma_start(wq_sb[:], wq[:])
        for b in range(8):
            emit_in_dma("q", 0, b, nc.sync)
        nc.sync.dma_start(wk_sb[:], wk[:])
        nc.sync.dma_start(bq_sb[:], bq2[:])
        nc.sync.dma_start(bk_sb[:], bk2[:])
        for b in range(8):
            emit_in_dma("k", 0, b, nc.scalar)
        nc.sync.dma_start(wv_sb[:], wv[:])
        nc.sync.dma_start(bv_sb[:], bvv[:])
        nc.sync.dma_start(wg_sb[:], wg[:])
        for b in range(8):
            emit_in_dma("v", 0, b, nc.sync)
        nc.sync.dma_start(id_sb[:], ident[:])
        nc.sync.dma_start(bg01_sb[:], bg01[:])
        nc.sync.dma_start(bg23_sb[:], bg23[:])
        nc.sync.dma_start(wfc0_sb[:], wfc[0:128, :])
        nc.sync.dma_start(wfc1_sb[:], wfc[128:256, :])
        for b in range(2):
            emit_in_dma("q", 1, b, nc.sync)
            emit_in_dma("k", 1, b, nc.sync)
            emit_in_dma("v", 1, b, nc.sync)

        # window 0 proj + G emitted directly (nothing else to overlap yet)
        for t in ("q", "k", "v"):
            cls, cell = make_proj_closures(t, 0)
            for c in cls:
                c()
            if t == "q":
                q0cell = cell
            if t == "v":
                for c in make_vpath_closures(0, cell):
                    c()
        make_g_closure(0, q0cell)()

        # ================= main pipeline =================
        push_window_feed(1)
        emit_window_attn(0)
        drain()

        push_window_feed(2)
        filler.extend(make_fc_closures(0))
        emit_window_attn(1)
        drain()

        push_window_feed(3)
        filler.extend(make_fc_closures(1))
        emit_window_attn(2)
        drain()

        filler.extend(make_fc_closures(2))
        emit_window_attn(3)
        drain()

        for c in make_fc_closures(3):
            c()


def shard_inputs(inputs):
    """full inputs -> list of 8 per-core in_maps (numpy, device layouts)"""
    f16 = np.float16
    f32 = np.float32
    q = np.asarray(inputs["q"], f32)[0]
    k = np.asarray(inputs["k"], f32)[0]
    v = np.asarray(inputs["v"], f32)[0]
    Wq = np.asarray(inputs["Wq"], f32)
    Wk = np.asarray(inputs["Wk"], f32)
    Wv = np.asarray(inputs["Wv"], f32)
    bq = np.asarray(inputs["bq"], f32)
    bk = np.asarray(inputs["bk"], f32)
    bv = np.asarray(inputs["bv"], f32)
    WG = np.asarray(inputs["WG"], f32)
    bG = np.asarray(inputs["bG"], f32)
    Wfc = np.asarray(inputs["Wfc"], f32)

    qT = np.ascontiguousarray(q.T.astype(f16))
    kT = np.ascontiguousarray(k.T.astype(f16))
    vT = np.ascontiguousarray(v.T.astype(f16))
    ident = np.eye(128, dtype=f32)

    def chunked(w):
        # [E, M] -> [128, 16*M]: e-chunk ec at cols [M*ec, M*ec+M)
        M = w.shape[1]
        return np.ascontiguousarray(
            w.reshape(16, 128, M).transpose(1, 0, 2).reshape(128, 16 * M))

    maps = []
    for h in range(HK):
        sl = slice(h * D, (h + 1) * D)
        wq_h = Wq[:, sl]
        wk_h = Wk[:, sl]
        wv_h = Wv[:, sl]
        m = {
            "qT": qT, "kT": kT, "vT": vT,
            "wq": chunked(np.concatenate([wq_h, wq_h], 1)).astype(f16),
            "wk": chunked(np.concatenate([wk_h, wk_h], 1)).astype(f16),
            "wv": chunked(wv_h).astype(f16),
            "bq2": np.concatenate([bq[sl], bq[sl]]).reshape(128, 1).copy(),
            "bk2": np.concatenate([bk[sl], bk[sl]]).reshape(128, 1).copy(),
            "bvv": bv[sl].reshape(64, 1).copy(),
            "wg": np.concatenate([WG[h], WG[h]], 0).astype(f16),  # [128, 256]
            "bg01": bG[h, 0:128].reshape(128, 1).copy(),
            "bg23": bG[h, 128:256].reshape(128, 1).copy(),
            "wfc": Wfc[h * 256:(h + 1) * 256, :].astype(f16),
            "ident": ident,
        }
        maps.append(m)
    return maps


_compiled = None
last_results = None


def get_compiled():
    global _compiled
    if _compiled is None:
        _compiled = build_program()
    return _compiled


def kernel(**inputs):
    global last_results
    nc = get_compiled()
    in_maps = shard_inputs(inputs)
    last_results = bass_utils.run_bass_kernel_spmd(
        nc, in_maps, core_ids=list(range(8)))
    bfc = np.asarray(inputs["bfc"], np.float32)
    acc = np.zeros((N, E), np.float64)
    for res in last_results.results:
        acc += res["out"].astype(np.float64)
    full = (acc + bfc[None, :].astype(np.float64)).astype(np.float32)
    return full.reshape(1, N, E)


# revision 19
# speedup vs baseline: 1.5408x; 1.0242x over previous
"""CompoundHeadAttention TRN2 kernel (v2 — software-pipelined schedule).

Full-input contract: kernel(**inputs) takes the unsharded tensors from
setup_inputs() and returns the full [1, 2048, 2048] float32 output.

Sharding (8 cores, tensor-parallel over the HK=8 kv heads):
  core h owns kv head h: its Wq/Wk/Wv column slice, its WG[h]/bG[h], and
  Wfc row-slice [h*256:(h+1)*256, :].  Each core computes its head's
  attention + its partial FC output [2048, 2048]; the host sums the 8
  partials and adds bfc (the "all-reduce" of the row-sharded FC).

v2 schedule (vs v1): the PE stream is kept dense — scores (ST) run two
chunks ahead of the PV consumer so the ACT exp latency never stalls the
in-order PE queue; projection/G/FC matmuls are interleaved into the
attention stream as filler at a per-window rate; softmax denominators
use reciprocal_approx_fast (0.7us vs 4us); input DMAs are batched
(2 e-chunks per instr) and issued from both the Sync and ACT queues.
"""

import os
import sys
from collections import deque

import numpy as np

if "/opt/trn_rl_repo" not in sys.path and os.path.isdir("/opt/trn_rl_repo"):
    sys.path.insert(0, "/opt/trn_rl_repo")

import concourse.bass as bass  # noqa: E402
import concourse.mybir as mybir  # noqa: E402
import concourse.tile as tile  # noqa: E402
from concourse import bacc  # noqa: E402
from concourse import bass_utils  # noqa: E402

F32 = mybir.dt.float32
F32R = mybir.dt.float32r
F16 = mybir.dt.float16
AF = mybir.ActivationFunctionType

N = 2048
E = 2048
HK = 8
D = 64
G = 4
NB = 4         # 512-wide n-windows
FILL_RATE = [6, 3, 2, 0]   # filler pops per attention chunk-slot, per window


def build_program():
    nc = bacc.Bacc("TRN2", target_bir_lowering=False, debug=False,
                   enable_asserts=False)

    qT = nc.dram_tensor("qT", [E, N], F16, kind="ExternalInput").ap()
    kT = nc.dram_tensor("kT", [E, N], F16, kind="ExternalInput").ap()
    vT = nc.dram_tensor("vT", [E, N], F16, kind="ExternalInput").ap()
    # weight chunk layout: [128, 16*M] — e-chunk ec occupies cols [M*ec, M*ec+M)
    wq = nc.dram_tensor("wq", [128, 16 * 128], F16, kind="ExternalInput").ap()
    wk = nc.dram_tensor("wk", [128, 16 * 128], F16, kind="ExternalInput").ap()
    wv = nc.dram_tensor("wv", [128, 16 * 64], F16, kind="ExternalInput").ap()
    bq2 = nc.dram_tensor("bq2", [128, 1], F32, kind="ExternalInput").ap()
    bk2 = nc.dram_tensor("bk2", [128, 1], F32, kind="ExternalInput").ap()
    bvv = nc.dram_tensor("bvv", [64, 1], F32, kind="ExternalInput").ap()
    wg = nc.dram_tensor("wg", [128, 256], F16, kind="ExternalInput").ap()
    bg01 = nc.dram_tensor("bg01", [128, 1], F32, kind="ExternalInput").ap()
    bg23 = nc.dram_tensor("bg23", [128, 1], F32, kind="ExternalInput").ap()
    wfc = nc.dram_tensor("wfc", [256, E], F16, kind="ExternalInput").ap()
    ident = nc.dram_tensor("ident", [128, 128], F32, kind="ExternalInput").ap()
    out = nc.dram_tensor("out", [N, E], F16, kind="ExternalOutput").ap()

    with tile.TileContext(nc) as tc:
        build_tile_kernel(tc, qT=qT, kT=kT, vT=vT, wq=wq, wk=wk, wv=wv,
                          bq2=bq2, bk2=bk2, bvv=bvv, wg=wg, bg01=bg01,
                          bg23=bg23, wfc=wfc, ident=ident, out=out)
    nc.compile()
    return nc


def build_tile_kernel(tc, *, qT, kT, vT, wq, wk, wv, bq2, bk2, bvv, wg,
                      bg01, bg23, wfc, ident, out):
    nc = tc.nc

    import contextlib
    ctx = contextlib.ExitStack()
    ctx.__enter__()
    cp = ctx.enter_context(tc.tile_pool(name="persist", bufs=1))

    def ptile(shape, dtype, name):
        return cp.tile(shape, dtype, tag=name, name=name)

    # ---- persistent constants / state in SBUF ----
    wq_sb = ptile([128, 16 * 128], F16, "wq_sb")
    wk_sb = ptile([128, 16 * 128], F16, "wk_sb")
    wv_sb = ptile([128, 16 * 64], F16, "wv_sb")
    wg_sb = ptile([128, 256], F16, "wg_sb")
    wfc0_sb = ptile([128, E], F16, "wfc0_sb")
    wfc1_sb = ptile([128, E], F16, "wfc1_sb")
    id_sb = ptile([128, 128], F32, "id_sb")
    bq_sb = ptile([128, 1], F32, "bq_sb")
    bk_sb = ptile([128, 1], F32, "bk_sb")
    bv_sb = ptile([64, 1], F32, "bv_sb")
    bg01_sb = ptile([128, 1], F32, "bg01_sb")
    bg23_sb = ptile([128, 1], F32, "bg23_sb")
    ones_sb = ptile([128, 1], F32, "ones_sb")
    warm_sb = ptile([1, 1], F32, "warm_sb")

    kt_w = [ptile([128, 512], F16, f"kt{j}") for j in range(NB)]
    vo_w = [ptile([128, 4 * 65], F32R, f"vo{j}") for j in range(NB)]

    with ctx:
        in_pool = ctx.enter_context(tc.tile_pool(name="in_pool", bufs=11))
        qt_pool = ctx.enter_context(tc.tile_pool(name="qt_pool", bufs=2))
        qg_pool = ctx.enter_context(tc.tile_pool(name="qg_pool", bufs=2))
        hid_pool = ctx.enter_context(tc.tile_pool(name="hid_pool", bufs=2))
        vt_pool = ctx.enter_context(tc.tile_pool(name="vt_pool", bufs=2))
        pt_pool = ctx.enter_context(tc.tile_pool(name="pt_pool", bufs=3))
        rec_pool = ctx.enter_context(tc.tile_pool(name="rec_pool", bufs=2))
        fco_pool = ctx.enter_context(tc.tile_pool(name="fco_pool", bufs=2))
        misc_ps = ctx.enter_context(
            tc.tile_pool(name="misc_ps", bufs=2, space="PSUM"))
        st_ps = ctx.enter_context(
            tc.tile_pool(name="st_ps", bufs=2, space="PSUM"))
        pv_ps = ctx.enter_context(
            tc.tile_pool(name="pv_ps", bufs=2, space="PSUM"))

        # ---------- shared state set as emission progresses ----------
        in_tiles = {}    # (tensor, pair, batch) -> sbuf tile [128, 2048]
        qg01_w = [None] * NB
        qg23_w = [None] * NB
        hid01_w = [None] * NB
        hid23_w = [None] * NB

        filler = deque()

        def fill(n):
            c = 0
            while filler and c < n:
                filler.popleft()()
                c += 1

        def drain():
            while filler:
                filler.popleft()()

        # ---------- DMA emission helpers ----------
        TSRC = {"q": (qT, "qin"), "k": (kT, "kin"), "v": (vT, "vin")}

        def emit_in_dma(t, P, b, eng, half=None):
            """half=None: full [128,2,1024] tile load.  half=0/1: load only
            that window's 512-col slice (allows the w0 slices to land first
            and the w1 slices to stream during window-0 attention)."""
            src_t, tag = TSRC[t]
            if half in (None, 0):
                ti = in_pool.tile([128, 2048], F16, tag=tag, name=f"{t}in")
                in_tiles[(t, P, b)] = ti
            else:
                ti = in_tiles[(t, P, b)]
            dst3 = ti[:].rearrange("p (c n) -> p c n", c=2)
            src3 = src_t[bass.ds(256 * b, 256),
                         bass.ds(P * 1024, 1024)].rearrange(
                             "(c p) n -> p c n", p=128)
            if half is None:
                eng.dma_start(dst3, src3)
            else:
                eng.dma_start(dst3[:, :, 512 * half: 512 * half + 512],
                              src3[:, :, 512 * half: 512 * half + 512])

        # ---------- projection emission (per tensor, per window) ----------
        def make_proj_closures(t, j):
            """16 matmuls (8 batch-closures) + 1 bias closure for tensor t,
            window j. Sets qt/kt/vt state."""
            P, h = j // 2, j % 2
            cell = {}

            def mk_mm(b):
                def go():
                    if b == 0:
                        rows = 64 if t == "v" else 128
                        cell["ps"] = misc_ps.tile([rows, 512], F32, tag="mm",
                                                  name=f"{t}_ps")
                    w_sb = {"q": wq_sb, "k": wk_sb, "v": wv_sb}[t]
                    M = 64 if t == "v" else 128
                    ps = cell["ps"]
                    for c in range(2):
                        ec = 2 * b + c
                        mv = in_tiles[(t, P, b)][
                            :, 1024 * c + 512 * h: 1024 * c + 512 * h + 512]
                        nc.tensor.matmul(ps[:], w_sb[:, bass.ts(ec, M)], mv,
                                         start=(ec == 0), stop=(ec == 15))
                return go

            def bias():
                ps = cell["ps"]
                if t == "q":
                    qt = qt_pool.tile([128, 512], F16, tag="qt", name="qt")
                    nc.scalar.activation(qt[:], ps[:], AF.Identity,
                                         bias=bq_sb[:])
                    cell["qt"] = qt
                elif t == "k":
                    nc.scalar.activation(kt_w[j][:], ps[:], AF.Identity,
                                         bias=bk_sb[:])
                else:
                    vt = vt_pool.tile([64, 512], F32, tag="vt", name="vt")
                    nc.scalar.activation(vt[:], ps[:], AF.Identity,
                                         bias=bv_sb[:])
                    cell["vt"] = vt

            return [mk_mm(b) for b in range(8)] + [bias], cell

        def make_vpath_closures(j, vcell):
            """PE transposes + DVE copies: VT -> vo_w[j] data columns."""
            tr_cell = {}

            def tr():
                tr_ps = misc_ps.tile([128, 256], F32, tag="mm", name="tr_ps")
                for t4 in range(4):
                    nc.tensor.transpose(tr_ps[:, bass.ts(t4, 64)],
                                        vcell["vt"][:, bass.ts(t4, 128)],
                                        id_sb[0:64, 0:64])
                tr_cell["tr"] = tr_ps

            def cp_out():
                vo3 = vo_w[j][:].rearrange("p (t c) -> p t c", c=65)
                for t4 in range(4):
                    nc.vector.tensor_copy(vo3[:, t4, 0:64],
                                          tr_cell["tr"][:, bass.ts(t4, 64)])

            return [tr, cp_out]

        def make_g_closure(j, qcell):
            def go():
                g01 = misc_ps.tile([128, 512], F32, tag="mm", name="g01_ps")
                nc.tensor.matmul(g01[:], wg_sb[0:64, 0:128],
                                 qcell["qt"][0:64, :], start=True, stop=True)
                g23 = misc_ps.tile([128, 512], F32, tag="mm", name="g23_ps")
                nc.tensor.matmul(g23[:], wg_sb[64:128, 128:256],
                                 qcell["qt"][64:128, :], start=True, stop=True)
                qg01 = qg_pool.tile([128, 512], F16, tag="qg01", name="qg01")
                qg23 = qg_pool.tile([128, 512], F16, tag="qg23", name="qg23")
                nc.scalar.activation(qg01[:], g01[:], AF.Identity,
                                     bias=bg01_sb[:])
                nc.scalar.activation(qg23[:], g23[:], AF.Identity,
                                     bias=bg23_sb[:])
                qg01_w[j] = qg01
                qg23_w[j] = qg23
            return go

        def push_window_feed(j):
            """Queue proj+G for window j as filler closures.  For j==1,
            interleave the pair-1 input DMAs right behind the proj closure
            that frees each input buffer."""
            def extend_interleaved(t, cls):
                mms, bias = cls[:8], cls[8]
                for b, mm in enumerate(mms):
                    filler.append(mm)
                    if j == 1 and b >= 3:
                        filler.append(
                            lambda t=t, b=b: emit_in_dma(t, 1, b, nc.sync))
                filler.append(bias)

            if j == 1:
                # second window-halves of the pair-0 inputs stream in as
                # filler on two DMA rings while window-0 attention runs
                for b in range(8):
                    filler.append(
                        lambda b=b: emit_in_dma("q", 0, b, nc.sync, half=1))
                for b in range(8):
                    filler.append(
                        lambda b=b: emit_in_dma("k", 0, b, nc.scalar, half=1))
                for b in range(8):
                    filler.append(
                        lambda b=b: emit_in_dma("v", 0, b, nc.sync, half=1))
            qcl, qcell = make_proj_closures("q", j)
            extend_interleaved("q", qcl)
            kcl, _ = make_proj_closures("k", j)
            extend_interleaved("k", kcl)
            vcl, vcell = make_proj_closures("v", j)
            extend_interleaved("v", vcl)
            filler.extend(make_vpath_closures(j, vcell))
            filler.append(make_g_closure(j, qcell))

        # ---------- FC emission ----------
        def make_fc_closures(j):
            cls = []
            for m in range(4):
                cell = {}
                for eo in range(4):
                    def go(m=m, eo=eo, cell=cell):
                        if eo == 0:
                            cell["stage"] = fco_pool.tile(
                                [128, 2048], F16, tag="stage", name="stage")
                        fc_ps = misc_ps.tile([128, 512], F32, tag="mm",
                                             name="fc_ps")
                        nc.tensor.matmul(fc_ps[:],
                                         hid01_w[j][:, bass.ts(m, 128)],
                                         wfc0_sb[:, bass.ts(eo, 512)],
                                         start=True, stop=False)
                        nc.tensor.matmul(fc_ps[:],
                                         hid23_w[j][:, bass.ts(m, 128)],
                                         wfc1_sb[:, bass.ts(eo, 512)],
                                         start=False, stop=True)
                        nc.vector.tensor_copy(
                            cell["stage"][:, bass.ts(eo, 512)], fc_ps[:])
                        rows = slice(512 * j + 128 * m, 512 * j + 128 * m + 128)
                        if j == 3:
                            # last window: DMA per-eo so the final transfer
                            # is small and the drain tail short
                            nc.sync.dma_start(
                                out[rows, bass.ts(eo, 512)],
                                cell["stage"][:, bass.ts(eo, 512)])
                        elif eo == 3:
                            nc.sync.dma_start(out[rows, :], cell["stage"][:])
                    cls.append(go)
            return cls

        # ---------- attention emission ----------
        def emit_window_attn(j):
            K = 4 * j + 4
            for pair, qg_of in ((0, qg01_w), (1, qg23_w)):
                qg = qg_of[j]
                pv_a = pv_ps.tile([65, 512], F32, tag="pv", name="pv_a")
                pv_b = pv_ps.tile([65, 512], F32, tag="pv", name="pv_b")
                pts = {}

                def st_step(k):
                    kt_c = kt_w[k // 4]
                    ks = bass.ts(k % 4, 128)
                    i = k - 4 * j
                    off = max(0, 128 * i)
                    if off == 384:
                        off = 256
                    st = st_ps.tile([128, 1024], F32, tag="st", name="st")
                    nc.tensor.matmul(st[:, off:512], kt_c[0:64, ks],
                                     qg[0:64, off:512], start=True, stop=True)
                    nc.tensor.matmul(st[:, 512 + off:1024], kt_c[64:128, ks],
                                     qg[64:128, off:512],
                                     start=True, stop=True)
                    pt = pt_pool.tile([128, 1024], F32R, tag="pt", name="pt")
                    st3 = st[:].rearrange("p (g c) -> p g c", c=512)
                    pt3 = pt[:].rearrange("p (g c) -> p g c", c=512)
                    nc.scalar.activation(pt3[:, :, off:512],
                                         st3[:, :, off:512],
                                         AF.Exp, scale=8.0)
                    if i >= 0:
                        mw = 128 * i + 128 - off
                        nc.gpsimd.affine_select(
                            out=pt3[:, :, off:off + mw],
                            in_=pt3[:, :, off:off + mw],
                            compare_op=mybir.AluOpType.is_ge,
                            fill=0.0, base=-(128 * i - off),
                            pattern=[[0, 2], [1, mw]],
                            channel_multiplier=-1)
                    pts[k] = (pt, off)

                def pv_step(k):
                    pt, off = pts.pop(k)
                    vo_c = vo_w[k // 4]
                    vsl = vo_c[:, (k % 4) * 65:(k % 4) * 65 + 65]
                    nc.tensor.matmul(pv_a[:, off:512], vsl, pt[:, off:512],
                                     start=(k == 0), stop=(k == K - 1))
                    nc.tensor.matmul(pv_b[:, off:512], vsl,
                                     pt[:, 512 + off:1024],
                                     start=(k == 0), stop=(k == K - 1))

                fill(2)
                st_step(0)
                if K > 1:
                    st_step(1)
                for k in range(K):
                    if k + 2 < K:
                        st_step(k + 2)
                    # extra filler at the pair start covers the previous
                    # pair's normalize chain before pv psum reuse
                    fill(FILL_RATE[j] + ((3 if j == 3 else 2) if k < 2 else 0))
                    pv_step(k)

                # normalize: hid[g-half] = pv[0:64] * (1/pv[64])
                if pair == 0:
                    hid = hid_pool.tile([128, 512], F16, tag="hid01",
                                        name="hid01")
                    hid01_w[j] = hid
                else:
                    hid = hid_pool.tile([128, 512], F16, tag="hid23",
                                        name="hid23")
                    hid23_w[j] = hid
                den_a = rec_pool.tile([1, 512], F32, tag="den", name="den_a")
                nc.vector.tensor_copy(den_a[:], pv_a[64:65, :])
                rec_a = rec_pool.tile([1, 512], F32, tag="rec", name="rec_a")
                nc.vector.reciprocal_approx_fast(rec_a[:], den_a[:])
                recr_a = rec_pool.tile([64, 512], F32, tag="recr",
                                       name="recr_a")
                nc.gpsimd.partition_broadcast(recr_a[:], rec_a[:])
                den_b = rec_pool.tile([1, 512], F32, tag="den", name="den_b")
                nc.vector.tensor_copy(den_b[:], pv_b[64:65, :])
                rec_b = rec_pool.tile([1, 512], F32, tag="rec", name="rec_b")
                nc.vector.reciprocal_approx_fast(rec_b[:], den_b[:])
                recr_b = rec_pool.tile([64, 512], F32, tag="recr",
                                       name="recr_b")
                nc.gpsimd.partition_broadcast(recr_b[:], rec_b[:])
                nc.vector.tensor_mul(hid[0:64, :], pv_a[0:64, :], recr_a[:])
                nc.vector.tensor_mul(hid[64:128, :], pv_b[0:64, :],
                                     recr_b[:])

        # ================= prologue =================
        nc.vector.memset(ones_sb[:], 1.0)
        nc.scalar.activation(warm_sb[:], ones_sb[0:1, :], AF.Exp, scale=1.0)
        for j in range(NB):
            for t4 in range(4):
                nc.vector.tensor_copy(
                    vo_w[j][:, t4 * 65 + 64: t4 * 65 + 65], ones_sb[:])

        # pair-0 window-0 halves first (2MB/ring): q on Sync, k/v on ACT ring
        nc.sync.dma_start(wq_sb[:], wq[:])
        for b in range(8):
            emit_in_dma("q", 0, b, nc.sync, half=0)
        for b in range(8):
            emit_in_dma("k", 0, b, nc.scalar, half=0)
        nc.sync.dma_start(wk_sb[:], wk[:])
        nc.sync.dma_start(bq_sb[:], bq2[:])
        nc.sync.dma_start(bk_sb[:], bk2[:])
        nc.sync.dma_start(wv_sb[:], wv[:])
        nc.sync.dma_start(bv_sb[:], bvv[:])
        nc.sync.dma_start(wg_sb[:], wg[:])
        for b in range(8):
            emit_in_dma("v", 0, b, nc.scalar, half=0)
        nc.sync.dma_start(id_sb[:], ident[:])
        nc.sync.dma_start(bg01_sb[:], bg01[:])
        nc.sync.dma_start(bg23_sb[:], bg23[:])
        nc.sync.dma_start(wfc0_sb[:], wfc[0:128, :])
        nc.sync.dma_start(wfc1_sb[:], wfc[128:256, :])
        for b in range(3):
            emit_in_dma("q", 1, b, nc.sync)
            emit_in_dma("k", 1, b, nc.sync)
            emit_in_dma("v", 1, b, nc.sync)

        # window 0 proj + G emitted directly (nothing else to overlap yet)
        for t in ("q", "k", "v"):
            cls, cell = make_proj_closures(t, 0)
            for c in cls:
                c()
            if t == "q":
                q0cell = cell
            if t == "v":
                for c in make_vpath_closures(0, cell):
                    c()
        make_g_closure(0, q0cell)()

        # ================= main pipeline =================
        push_window_feed(1)
        emit_window_attn(0)
        drain()

        push_window_feed(2)
        filler.extend(make_fc_closures(0))
        emit_window_attn(1)
        drain()

        push_window_feed(3)
        filler.extend(make_fc_closures(1))
        emit_window_attn(2)
        drain()

        filler.extend(make_fc_closures(2))
        emit_window_attn(3)
        drain()

        for c in make_fc_closures(3):
            c()


def shard_inputs(inputs):
    """full inputs -> list of 8 per-core in_maps (numpy, device layouts)"""
    f16 = np.float16
    f32 = np.float32
    q = np.asarray(inputs["q"], f32)[0]
    k = np.asarray(inputs["k"], f32)[0]
    v = np.asarray(inputs["v"], f32)[0]
    Wq = np.asarray(inputs["Wq"], f32)
    Wk = np.asarray(inputs["Wk"], f32)
    Wv = np.asarray(inputs["Wv"], f32)
    bq = np.asarray(inputs["bq"], f32)
    bk = np.asarray(inputs["bk"], f32)
    bv = np.asarray(inputs["bv"], f32)
    WG = np.asarray(inputs["WG"], f32)
    bG = np.asarray(inputs["bG"], f32)
    Wfc = np.asarray(inputs["Wfc"], f32)

    qT = np.ascontiguousarray(q.T.astype(f16))
    kT = np.ascontiguousarray(k.T.astype(f16))
    vT = np.ascontiguousarray(v.T.astype(f16))
    ident = np.eye(128, dtype=f32)

    def chunked(w):
        # [E, M] -> [128, 16*M]: e-chunk ec at cols [M*ec, M*ec+M)
        M = w.shape[1]
        return np.ascontiguousarray(
            w.reshape(16, 128, M).transpose(1, 0, 2).reshape(128, 16 * M))

    maps = []
    for h in range(HK):
        sl = slice(h * D, (h + 1) * D)
        wq_h = Wq[:, sl]
        wk_h = Wk[:, sl]
        wv_h = Wv[:, sl]
        m = {
            "qT": qT, "kT": kT, "vT": vT,
            "wq": chunked(np.concatenate([wq_h, wq_h], 1)).astype(f16),
            "wk": chunked(np.concatenate([wk_h, wk_h], 1)).astype(f16),
            "wv": chunked(wv_h).astype(f16),
            "bq2": np.concatenate([bq[sl], bq[sl]]).reshape(128, 1).copy(),
            "bk2": np.concatenate([bk[sl], bk[sl]]).reshape(128, 1).copy(),
            "bvv": bv[sl].reshape(64, 1).copy(),
            "wg": np.concatenate([WG[h], WG[h]], 0).astype(f16),  # [128, 256]
            "bg01": bG[h, 0:128].reshape(128, 1).copy(),
            "bg23": bG[h, 128:256].reshape(128, 1).copy(),
            "wfc": Wfc[h * 256:(h + 1) * 256, :].astype(f16),
            "ident": ident,
        }
        maps.append(m)
    return maps


_compiled = None
last_results = None


def get_compiled():
    global _compiled
    if _compiled is None:
        _compiled = build_program()
    return _compiled


def kernel(**inputs):
    global last_results
    nc = get_compiled()
    in_maps = shard_inputs(inputs)
    last_results = bass_utils.run_bass_kernel_spmd(
        nc, in_maps, core_ids=list(range(8)))
    bfc = np.asarray(inputs["bfc"], np.float32)
    acc = np.zeros((N, E), np.float64)
    for res in last_results.results:
        acc += res["out"].astype(np.float64)
    full = (acc + bfc[None, :].astype(np.float64)).astype(np.float32)
    return full.reshape(1, N, E)


# revision 21
# speedup vs baseline: 1.5768x; 1.0233x over previous
"""CompoundHeadAttention TRN2 kernel (v2 — software-pipelined schedule).

Full-input contract: kernel(**inputs) takes the unsharded tensors from
setup_inputs() and returns the full [1, 2048, 2048] float32 output.

Sharding (8 cores, tensor-parallel over the HK=8 kv heads):
  core h owns kv head h: its Wq/Wk/Wv column slice, its WG[h]/bG[h], and
  Wfc row-slice [h*256:(h+1)*256, :].  Each core computes its head's
  attention + its partial FC output [2048, 2048]; the host sums the 8
  partials and adds bfc (the "all-reduce" of the row-sharded FC).

v2 schedule (vs v1): the PE stream is kept dense — scores (ST) run two
chunks ahead of the PV consumer so the ACT exp latency never stalls the
in-order PE queue; projection/G/FC matmuls are interleaved into the
attention stream as filler at a per-window rate; softmax denominators
use reciprocal_approx_fast (0.7us vs 4us); input DMAs are batched
(2 e-chunks per instr) and issued from both the Sync and ACT queues.
"""

import os
import sys
from collections import deque

import numpy as np

if "/opt/trn_rl_repo" not in sys.path and os.path.isdir("/opt/trn_rl_repo"):
    sys.path.insert(0, "/opt/trn_rl_repo")

import concourse.bass as bass  # noqa: E402
import concourse.mybir as mybir  # noqa: E402
import concourse.tile as tile  # noqa: E402
from concourse import bacc  # noqa: E402
from concourse import bass_utils  # noqa: E402

F32 = mybir.dt.float32
F32R = mybir.dt.float32r
F16 = mybir.dt.float16
AF = mybir.ActivationFunctionType

N = 2048
E = 2048
HK = 8
D = 64
G = 4
NB = 4         # 512-wide n-windows
FILL_RATE = [6, 3, 2, 0]   # filler pops per attention chunk-slot, per window


def build_program():
    nc = bacc.Bacc("TRN2", target_bir_lowering=False, debug=False,
                   enable_asserts=False)

    qT = nc.dram_tensor("qT", [E, N], F16, kind="ExternalInput").ap()
    kT = nc.dram_tensor("kT", [E, N], F16, kind="ExternalInput").ap()
    vT = nc.dram_tensor("vT", [E, N], F16, kind="ExternalInput").ap()
    # weight chunk layout: [128, 16*M] — e-chunk ec occupies cols [M*ec, M*ec+M)
    wq = nc.dram_tensor("wq", [128, 16 * 128], F16, kind="ExternalInput").ap()
    wk = nc.dram_tensor("wk", [128, 16 * 128], F16, kind="ExternalInput").ap()
    wv = nc.dram_tensor("wv", [128, 16 * 64], F16, kind="ExternalInput").ap()
    bq2 = nc.dram_tensor("bq2", [128, 1], F32, kind="ExternalInput").ap()
    bk2 = nc.dram_tensor("bk2", [128, 1], F32, kind="ExternalInput").ap()
    bvv = nc.dram_tensor("bvv", [64, 1], F32, kind="ExternalInput").ap()
    wg = nc.dram_tensor("wg", [128, 256], F16, kind="ExternalInput").ap()
    bg01 = nc.dram_tensor("bg01", [128, 1], F32, kind="ExternalInput").ap()
    bg23 = nc.dram_tensor("bg23", [128, 1], F32, kind="ExternalInput").ap()
    wfc = nc.dram_tensor("wfc", [256, E], F16, kind="ExternalInput").ap()
    ident = nc.dram_tensor("ident", [128, 128], F32, kind="ExternalInput").ap()
    out = nc.dram_tensor("out", [N, E], F16, kind="ExternalOutput").ap()

    with tile.TileContext(nc) as tc:
        build_tile_kernel(tc, qT=qT, kT=kT, vT=vT, wq=wq, wk=wk, wv=wv,
                          bq2=bq2, bk2=bk2, bvv=bvv, wg=wg, bg01=bg01,
                          bg23=bg23, wfc=wfc, ident=ident, out=out)
    nc.compile()
    return nc


def build_tile_kernel(tc, *, qT, kT, vT, wq, wk, wv, bq2, bk2, bvv, wg,
                      bg01, bg23, wfc, ident, out):
    nc = tc.nc

    import contextlib
    ctx = contextlib.ExitStack()
    ctx.__enter__()
    cp = ctx.enter_context(tc.tile_pool(name="persist", bufs=1))

    def ptile(shape, dtype, name):
        return cp.tile(shape, dtype, tag=name, name=name)

    # ---- persistent constants / state in SBUF ----
    wq_sb = ptile([128, 16 * 128], F16, "wq_sb")
    wk_sb = ptile([128, 16 * 128], F16, "wk_sb")
    wv_sb = ptile([128, 16 * 64], F16, "wv_sb")
    wg_sb = ptile([128, 256], F16, "wg_sb")
    wfc0_sb = ptile([128, E], F16, "wfc0_sb")
    wfc1_sb = ptile([128, E], F16, "wfc1_sb")
    id_sb = ptile([128, 128], F32, "id_sb")
    bq_sb = ptile([128, 1], F32, "bq_sb")
    bk_sb = ptile([128, 1], F32, "bk_sb")
    bv_sb = ptile([64, 1], F32, "bv_sb")
    bg01_sb = ptile([128, 1], F32, "bg01_sb")
    bg23_sb = ptile([128, 1], F32, "bg23_sb")
    ones_sb = ptile([128, 1], F32, "ones_sb")
    warm_sb = ptile([1, 1], F32, "warm_sb")

    kt_w = [ptile([128, 512], F16, f"kt{j}") for j in range(NB)]
    vo_w = [ptile([128, 4 * 65], F32R, f"vo{j}") for j in range(NB)]

    with ctx:
        in_pool = ctx.enter_context(tc.tile_pool(name="in_pool", bufs=11))
        qt_pool = ctx.enter_context(tc.tile_pool(name="qt_pool", bufs=2))
        qg_pool = ctx.enter_context(tc.tile_pool(name="qg_pool", bufs=2))
        hid_pool = ctx.enter_context(tc.tile_pool(name="hid_pool", bufs=2))
        vt_pool = ctx.enter_context(tc.tile_pool(name="vt_pool", bufs=2))
        pt_pool = ctx.enter_context(tc.tile_pool(name="pt_pool", bufs=3))
        rec_pool = ctx.enter_context(tc.tile_pool(name="rec_pool", bufs=2))
        fco_pool = ctx.enter_context(tc.tile_pool(name="fco_pool", bufs=2))
        misc_ps = ctx.enter_context(
            tc.tile_pool(name="misc_ps", bufs=2, space="PSUM"))
        st_ps = ctx.enter_context(
            tc.tile_pool(name="st_ps", bufs=2, space="PSUM"))
        pv_ps = ctx.enter_context(
            tc.tile_pool(name="pv_ps", bufs=2, space="PSUM"))

        # ---------- shared state set as emission progresses ----------
        in_tiles = {}    # (tensor, pair, batch) -> sbuf tile [128, 2048]
        qg01_w = [None] * NB
        qg23_w = [None] * NB
        hid01_w = [None] * NB
        hid23_w = [None] * NB

        filler = deque()

        def fill(n):
            c = 0
            while filler and c < n:
                filler.popleft()()
                c += 1

        def drain():
            while filler:
                filler.popleft()()

        # ---------- DMA emission helpers ----------
        TSRC = {"q": (qT, "qin"), "k": (kT, "kin"), "v": (vT, "vin")}

        def emit_in_dma(t, P, b, eng, half=None):
            """half=None: full [128,2,1024] tile load.  half=0/1: load only
            that window's 512-col slice (allows the w0 slices to land first
            and the w1 slices to stream during window-0 attention)."""
            src_t, tag = TSRC[t]
            if half in (None, 0):
                ti = in_pool.tile([128, 2048], F16, tag=tag, name=f"{t}in")
                in_tiles[(t, P, b)] = ti
            else:
                ti = in_tiles[(t, P, b)]
            dst3 = ti[:].rearrange("p (c n) -> p c n", c=2)
            src3 = src_t[bass.ds(256 * b, 256),
                         bass.ds(P * 1024, 1024)].rearrange(
                             "(c p) n -> p c n", p=128)
            if half is None:
                eng.dma_start(dst3, src3)
            else:
                eng.dma_start(dst3[:, :, 512 * half: 512 * half + 512],
                              src3[:, :, 512 * half: 512 * half + 512])

        # ---------- projection emission (per tensor, per window) ----------
        def make_proj_closures(t, j):
            """16 matmuls (8 batch-closures) + 1 bias closure for tensor t,
            window j. Sets qt/kt/vt state."""
            P, h = j // 2, j % 2
            cell = {}

            def mk_mm(b):
                def go():
                    if b == 0:
                        rows = 64 if t == "v" else 128
                        cell["ps"] = misc_ps.tile([rows, 512], F32, tag="mm",
                                                  name=f"{t}_ps")
                    w_sb = {"q": wq_sb, "k": wk_sb, "v": wv_sb}[t]
                    M = 64 if t == "v" else 128
                    ps = cell["ps"]
                    for c in range(2):
                        ec = 2 * b + c
                        mv = in_tiles[(t, P, b)][
                            :, 1024 * c + 512 * h: 1024 * c + 512 * h + 512]
                        nc.tensor.matmul(ps[:], w_sb[:, bass.ts(ec, M)], mv,
                                         start=(ec == 0), stop=(ec == 15))
                return go

            def bias():
                ps = cell["ps"]
                if t == "q":
                    qt = qt_pool.tile([128, 512], F16, tag="qt", name="qt")
                    nc.scalar.activation(qt[:], ps[:], AF.Identity,
                                         bias=bq_sb[:])
                    cell["qt"] = qt
                elif t == "k":
                    nc.scalar.activation(kt_w[j][:], ps[:], AF.Identity,
                                         bias=bk_sb[:])
                else:
                    vt = vt_pool.tile([64, 512], F32, tag="vt", name="vt")
                    nc.scalar.activation(vt[:], ps[:], AF.Identity,
                                         bias=bv_sb[:])
                    cell["vt"] = vt

            return [mk_mm(b) for b in range(8)] + [bias], cell

        def make_vpath_closures(j, vcell):
            """PE transposes + DVE copies: VT -> vo_w[j] data columns."""
            tr_cell = {}

            def tr():
                tr_ps = misc_ps.tile([128, 256], F32, tag="mm", name="tr_ps")
                for t4 in range(4):
                    nc.tensor.transpose(tr_ps[:, bass.ts(t4, 64)],
                                        vcell["vt"][:, bass.ts(t4, 128)],
                                        id_sb[0:64, 0:64])
                tr_cell["tr"] = tr_ps

            def cp_out():
                vo3 = vo_w[j][:].rearrange("p (t c) -> p t c", c=65)
                for t4 in range(4):
                    nc.vector.tensor_copy(vo3[:, t4, 0:64],
                                          tr_cell["tr"][:, bass.ts(t4, 64)])

            return [tr, cp_out]

        def make_g_closure(j, qcell):
            def go():
                g01 = misc_ps.tile([128, 512], F32, tag="mm", name="g01_ps")
                nc.tensor.matmul(g01[:], wg_sb[0:64, 0:128],
                                 qcell["qt"][0:64, :], start=True, stop=True)
                g23 = misc_ps.tile([128, 512], F32, tag="mm", name="g23_ps")
                nc.tensor.matmul(g23[:], wg_sb[64:128, 128:256],
                                 qcell["qt"][64:128, :], start=True, stop=True)
                qg01 = qg_pool.tile([128, 512], F16, tag="qg01", name="qg01")
                qg23 = qg_pool.tile([128, 512], F16, tag="qg23", name="qg23")
                nc.scalar.activation(qg01[:], g01[:], AF.Identity,
                                     bias=bg01_sb[:])
                nc.scalar.activation(qg23[:], g23[:], AF.Identity,
                                     bias=bg23_sb[:])
                qg01_w[j] = qg01
                qg23_w[j] = qg23
            return go

        def push_window_feed(j):
            """Queue proj+G for window j as filler closures.  For j==1,
            interleave the pair-1 input DMAs right behind the proj closure
            that frees each input buffer."""
            def extend_interleaved(t, cls):
                mms, bias = cls[:8], cls[8]
                for b, mm in enumerate(mms):
                    filler.append(mm)
                filler.append(bias)

            if j == 1:
                # second window-halves of the pair-0 inputs stream in as
                # filler on two DMA rings while window-0 attention runs
                for b in range(8):
                    filler.append(
                        lambda b=b: emit_in_dma("q", 0, b, nc.sync, half=1))
                for b in range(8):
                    filler.append(
                        lambda b=b: emit_in_dma("k", 0, b, nc.scalar, half=1))
                for b in range(8):
                    filler.append(
                        lambda b=b: emit_in_dma("v", 0, b, nc.sync, half=1))
                for t in ("q", "k", "v"):
                    for b in range(3, 8):
                        filler.append(
                            lambda t=t, b=b: emit_in_dma(t, 1, b, nc.sync))
            qcl, qcell = make_proj_closures("q", j)
            extend_interleaved("q", qcl)
            kcl, _ = make_proj_closures("k", j)
            extend_interleaved("k", kcl)
            vcl, vcell = make_proj_closures("v", j)
            extend_interleaved("v", vcl)
            filler.extend(make_vpath_closures(j, vcell))
            filler.append(make_g_closure(j, qcell))

        # ---------- FC emission ----------
        def make_fc_closures(j):
            cls = []
            for m in range(4):
                cell = {}
                for eo in range(4):
                    def go(m=m, eo=eo, cell=cell):
                        if eo == 0:
                            cell["stage"] = fco_pool.tile(
                                [128, 2048], F16, tag="stage", name="stage")
                        if j == 3 and (m * 4 + eo) % 2 == 1:
                            fc_ps = st_ps.tile([128, 512], F32, tag="st",
                                               name="fc_ps")
                        else:
                            fc_ps = misc_ps.tile([128, 512], F32, tag="mm",
                                                 name="fc_ps")
                        nc.tensor.matmul(fc_ps[:],
                                         hid01_w[j][:, bass.ts(m, 128)],
                                         wfc0_sb[:, bass.ts(eo, 512)],
                                         start=True, stop=False)
                        nc.tensor.matmul(fc_ps[:],
                                         hid23_w[j][:, bass.ts(m, 128)],
                                         wfc1_sb[:, bass.ts(eo, 512)],
                                         start=False, stop=True)
                        nc.vector.tensor_copy(
                            cell["stage"][:, bass.ts(eo, 512)], fc_ps[:])
                        rows = slice(512 * j + 128 * m, 512 * j + 128 * m + 128)
                        if j == 3:
                            # last window: DMA per-eo on alternating rings so
                            # the final transfers are small and parallel
                            eng = nc.scalar if eo % 2 else nc.sync
                            eng.dma_start(
                                out[rows, bass.ts(eo, 512)],
                                cell["stage"][:, bass.ts(eo, 512)])
                        elif eo == 3:
                            nc.sync.dma_start(out[rows, :], cell["stage"][:])
                    cls.append(go)
            return cls

        # ---------- attention emission ----------
        def emit_window_attn(j):
            K = 4 * j + 4
            for pair, qg_of in ((0, qg01_w), (1, qg23_w)):
                qg = qg_of[j]
                pv_a = pv_ps.tile([65, 512], F32, tag="pv", name="pv_a")
                pv_b = pv_ps.tile([65, 512], F32, tag="pv", name="pv_b")
                pts = {}

                def st_step(k):
                    kt_c = kt_w[k // 4]
                    ks = bass.ts(k % 4, 128)
                    i = k - 4 * j
                    off = max(0, 128 * i)
                    if off == 384:
                        off = 256
                    st = st_ps.tile([128, 1024], F32, tag="st", name="st")
                    nc.tensor.matmul(st[:, off:512], kt_c[0:64, ks],
                                     qg[0:64, off:512], start=True, stop=True)
                    nc.tensor.matmul(st[:, 512 + off:1024], kt_c[64:128, ks],
                                     qg[64:128, off:512],
                                     start=True, stop=True)
                    pt = pt_pool.tile([128, 1024], F32R, tag="pt", name="pt")
                    st3 = st[:].rearrange("p (g c) -> p g c", c=512)
                    pt3 = pt[:].rearrange("p (g c) -> p g c", c=512)
                    nc.scalar.activation(pt3[:, :, off:512],
                                         st3[:, :, off:512],
                                         AF.Exp, scale=8.0)
                    if i >= 0:
                        mw = 128 * i + 128 - off
                        nc.gpsimd.affine_select(
                            out=pt3[:, :, off:off + mw],
                            in_=pt3[:, :, off:off + mw],
                            compare_op=mybir.AluOpType.is_ge,
                            fill=0.0, base=-(128 * i - off),
                            pattern=[[0, 2], [1, mw]],
                            channel_multiplier=-1)
                    pts[k] = (pt, off)

                def pv_step(k):
                    pt, off = pts.pop(k)
                    vo_c = vo_w[k // 4]
                    vsl = vo_c[:, (k % 4) * 65:(k % 4) * 65 + 65]
                    nc.tensor.matmul(pv_a[:, off:512], vsl, pt[:, off:512],
                                     start=(k == 0), stop=(k == K - 1))
                    nc.tensor.matmul(pv_b[:, off:512], vsl,
                                     pt[:, 512 + off:1024],
                                     start=(k == 0), stop=(k == K - 1))

                fill(3 if j == 3 else 2)
                st_step(0)
                if K > 1:
                    st_step(1)
                for k in range(K):
                    if k + 2 < K:
                        st_step(k + 2)
                    # extra filler at the pair start covers the previous
                    # pair's normalize chain before pv psum reuse
                    fill(FILL_RATE[j] + ((3 if k < 1 else (2 if k < 4 else 0)) if j == 3 else (2 if k < 2 else 0)))
                    pv_step(k)

                # normalize: hid[g-half] = pv[0:64] * (1/pv[64])
                if pair == 0:
                    hid = hid_pool.tile([128, 512], F16, tag="hid01",
                                        name="hid01")
                    hid01_w[j] = hid
                else:
                    hid = hid_pool.tile([128, 512], F16, tag="hid23",
                                        name="hid23")
                    hid23_w[j] = hid
                den_a = rec_pool.tile([1, 512], F32, tag="den", name="den_a")
                nc.vector.tensor_copy(den_a[:], pv_a[64:65, :])
                rec_a = rec_pool.tile([1, 512], F32, tag="rec", name="rec_a")
                nc.vector.reciprocal_approx_fast(rec_a[:], den_a[:])
                recr_a = rec_pool.tile([64, 512], F32, tag="recr",
                                       name="recr_a")
                nc.gpsimd.partition_broadcast(recr_a[:], rec_a[:])
                den_b = rec_pool.tile([1, 512], F32, tag="den", name="den_b")
                nc.vector.tensor_copy(den_b[:], pv_b[64:65, :])
                rec_b = rec_pool.tile([1, 512], F32, tag="rec", name="rec_b")
                nc.vector.reciprocal_approx_fast(rec_b[:], den_b[:])
                recr_b = rec_pool.tile([64, 512], F32, tag="recr",
                                       name="recr_b")
                nc.gpsimd.partition_broadcast(recr_b[:], rec_b[:])
                nc.vector.tensor_mul(hid[0:64, :], pv_a[0:64, :], recr_a[:])
                nc.vector.tensor_mul(hid[64:128, :], pv_b[0:64, :],
                                     recr_b[:])

        # ================= prologue =================
        nc.vector.memset(ones_sb[:], 1.0)
        nc.scalar.activation(warm_sb[:], ones_sb[0:1, :], AF.Exp, scale=1.0)
        for j in range(NB):
            for t4 in range(4):
                nc.vector.tensor_copy(
                    vo_w[j][:, t4 * 65 + 64: t4 * 65 + 65], ones_sb[:])

        # pair-0 window-0 halves first (2MB/ring): q on Sync, k/v on ACT ring
        emit_in_dma("q", 0, 0, nc.sync, half=0)
        nc.sync.dma_start(wq_sb[:], wq[:])
        for b in range(1, 8):
            emit_in_dma("q", 0, b, nc.sync, half=0)
        for b in range(8):
            emit_in_dma("k", 0, b, nc.scalar, half=0)
        nc.sync.dma_start(wk_sb[:], wk[:])
        nc.sync.dma_start(bq_sb[:], bq2[:])
        nc.sync.dma_start(bk_sb[:], bk2[:])
        nc.sync.dma_start(wv_sb[:], wv[:])
        nc.sync.dma_start(bv_sb[:], bvv[:])
        nc.sync.dma_start(wg_sb[:], wg[:])
        for b in range(8):
            emit_in_dma("v", 0, b, nc.scalar, half=0)
        nc.sync.dma_start(id_sb[:], ident[:])
        nc.sync.dma_start(bg01_sb[:], bg01[:])
        nc.sync.dma_start(bg23_sb[:], bg23[:])
        nc.sync.dma_start(wfc0_sb[:], wfc[0:128, :])
        nc.sync.dma_start(wfc1_sb[:], wfc[128:256, :])
        for b in range(3):
            emit_in_dma("q", 1, b, nc.sync)
            emit_in_dma("k", 1, b, nc.sync)
            emit_in_dma("v", 1, b, nc.sync)

        # window 0 proj + G emitted directly (nothing else to overlap yet)
        for t in ("q", "k", "v"):
            cls, cell = make_proj_closures(t, 0)
            for c in cls:
                c()
            if t == "q":
                q0cell = cell
            if t == "v":
                for c in make_vpath_closures(0, cell):
                    c()
        make_g_closure(0, q0cell)()

        def zip_feed(fc_cls, feed_j):
            """Interleave FC m-groups between whole proj-tensor blocks (a
            projection's psum accumulation must not be interleaved with FC
            psum allocations — both rotate the same "mm" tag)."""
            qcl, qcell = make_proj_closures("q", feed_j)
            kcl, _ = make_proj_closures("k", feed_j)
            vcl, vcell = make_proj_closures("v", feed_j)
            fc_groups = [fc_cls[i:i + 4] for i in range(0, len(fc_cls), 4)]

            def grp(i):
                return fc_groups[i] if i < len(fc_groups) else []

            filler.extend(qcl)
            filler.extend(grp(0))
            filler.extend(kcl)
            filler.extend(grp(1))
            filler.extend(vcl)
            filler.extend(make_vpath_closures(feed_j, vcell))
            filler.extend(grp(2))
            filler.append(make_g_closure(feed_j, qcell))
            for g in fc_groups[3:]:
                filler.extend(g)

        # ================= main pipeline =================
        push_window_feed(1)
        emit_window_attn(0)
        drain()

        fc0 = make_fc_closures(0)
        zip_feed(fc0, 2)
        emit_window_attn(1)
        drain()

        fc1 = make_fc_closures(1)
        zip_feed(fc1[:12], 3)
        emit_window_attn(2)
        drain()

        filler.extend(fc1[12:])
        filler.extend(make_fc_closures(2))
        emit_window_attn(3)
        drain()

        for c in make_fc_closures(3):
            c()


def shard_inputs(inputs):
    """full inputs -> list of 8 per-core in_maps (numpy, device layouts)"""
    f16 = np.float16
    f32 = np.float32
    q = np.asarray(inputs["q"], f32)[0]
    k = np.asarray(inputs["k"], f32)[0]
    v = np.asarray(inputs["v"], f32)[0]
    Wq = np.asarray(inputs["Wq"], f32)
    Wk = np.asarray(inputs["Wk"], f32)
    Wv = np.asarray(inputs["Wv"], f32)
    bq = np.asarray(inputs["bq"], f32)
    bk = np.asarray(inputs["bk"], f32)
    bv = np.asarray(inputs["bv"], f32)
    WG = np.asarray(inputs["WG"], f32)
    bG = np.asarray(inputs["bG"], f32)
    Wfc = np.asarray(inputs["Wfc"], f32)

    qT = np.ascontiguousarray(q.T.astype(f16))
    kT = np.ascontiguousarray(k.T.astype(f16))
    vT = np.ascontiguousarray(v.T.astype(f16))
    ident = np.eye(128, dtype=f32)

    def chunked(w):
        # [E, M] -> [128, 16*M]: e-chunk ec at cols [M*ec, M*ec+M)
        M = w.shape[1]
        return np.ascontiguousarray(
            w.reshape(16, 128, M).transpose(1, 0, 2).reshape(128, 16 * M))

    maps = []
    for h in range(HK):
        sl = slice(h * D, (h + 1) * D)
        wq_h = Wq[:, sl]
        wk_h = Wk[:, sl]
        wv_h = Wv[:, sl]
        m = {
            "qT": qT, "kT": kT, "vT": vT,
            "wq": chunked(np.concatenate([wq_h, wq_h], 1)).astype(f16),
            "wk": chunked(np.concatenate([wk_h, wk_h], 1)).astype(f16),
            "wv": chunked(wv_h).astype(f16),
            "bq2": np.concatenate([bq[sl], bq[sl]]).reshape(128, 1).copy(),
            "bk2": np.concatenate([bk[sl], bk[sl]]).reshape(128, 1).copy(),
            "bvv": bv[sl].reshape(64, 1).copy(),
            "wg": np.concatenate([WG[h], WG[h]], 0).astype(f16),  # [128, 256]
            "bg01": bG[h, 0:128].reshape(128, 1).copy(),
            "bg23": bG[h, 128:256].reshape(128, 1).copy(),
            "wfc": Wfc[h * 256:(h + 1) * 256, :].astype(f16),
            "ident": ident,
        }
        maps.append(m)
    return maps


_compiled = None
last_results = None


def get_compiled():
    global _compiled
    if _compiled is None:
        _compiled = build_program()
    return _compiled


def kernel(**inputs):
    global last_results
    nc = get_compiled()
    in_maps = shard_inputs(inputs)
    last_results = bass_utils.run_bass_kernel_spmd(
        nc, in_maps, core_ids=list(range(8)))
    bfc = np.asarray(inputs["bfc"], np.float32)
    acc = np.zeros((N, E), np.float64)
    for res in last_results.results:
        acc += res["out"].astype(np.float64)
    full = (acc + bfc[None, :].astype(np.float64)).astype(np.float32)
    return full.reshape(1, N, E)
